# revision 1
# baseline (speedup 1.0000x reference)
"""GAT block (GATConv + InstanceNorm + residual + ELU) on 8 Trainium2 NeuronCores.

Strategy (graph/data parallel over dst nodes):
  - Host routes each edge to the core owning its dst node; per core, dst
    nodes are sorted by (degree, src<HALF-degree) and grouped into tiles of
    128 (dst node == partition, so aggregation needs no scatter).
  - Incoming edges of a tile live in padded slot columns: k=0 is the self
    loop (filled from on-chip hx_own, no gather), then group-A slots
    (src < HALF) and group-B slots (src >= HALF).
  - Slot rows are fetched with ONE batched dma_gather ucode instruction per
    (tile, half) from a [N+2, 192]-f32 table hx192 = x @ [W|w_src|w_dst|0]
    that each core builds locally (int16 gather indices fit because each
    half has < 32768 rows; row 0 / row HALF+1 are -1e30 dummy rows that
    softmax kills, used for padding slots).
  - Softmax over slots skips the segment max (logits are bounded, exp is
    clamped at -88 so it cannot overflow; result is mathematically equal).
  - a_edge = edge_attr @ v (v folded on host) via TensorE on a
    host-transposed 4-slot-interleaved eaT4 layout; the self loop's a_edge
    is (sum_k a_edge_k) / deg (linearity in edge_attr).
  - InstanceNorm stats via ones-matmul partition reduction, AllReduce'd
    across the 8 cores; finalize = per-channel affine + residual + ELU.
"""

import math
import numpy as np

P = 128


def _cfg_full():
    return dict(N=50000, E=1600000, F=128, H=8, Dh=16, ED=16, NC=8)


def _half(N):
    # multiple of 128 so Phase-A chunks never span the A/B table boundary;
    # both halves must stay < 32768 rows (int16 gather indices).
    h = ((N // 2) // P) * P
    assert h <= 32767 and (N - h) <= 32766
    return h


def _fold_weights(W, att_src, att_dst, W_e, att_edge, H, Dh, FX):
    F = W.shape[0]
    w_src = np.stack(
        [W[:, h * Dh:(h + 1) * Dh] @ att_src[h] for h in range(H)], axis=1)
    w_dst = np.stack(
        [W[:, h * Dh:(h + 1) * Dh] @ att_dst[h] for h in range(H)], axis=1)
    Wb = np.zeros((F, FX), dtype=np.float32)
    Wb[:, :F] = W
    Wb[:, F:F + H] = w_src
    Wb[:, F + H:F + 2 * H] = w_dst
    v = np.stack(
        [W_e[:, h * Dh:(h + 1) * Dh] @ att_edge[h] for h in range(H)], axis=1
    ).astype(np.float32)
    ED = W_e.shape[0]
    v4 = np.zeros((4 * ED, 4 * H), dtype=np.float32)
    for j in range(4):
        v4[j * ED:(j + 1) * ED, j * H:(j + 1) * H] = v
    return Wb, v4


def _preprocess(edge_index, edge_attr, cfg):
    N, ED, NC = cfg["N"], cfg["ED"], cfg["NC"]
    HALF = _half(N)
    Np = N // NC
    n_tiles = math.ceil(Np / P)
    src = np.asarray(edge_index[0]).astype(np.int64)
    dst = np.asarray(edge_index[1]).astype(np.int64)
    ea = np.asarray(edge_attr, dtype=np.float32)

    cores = []
    for c in range(NC):
        m = (dst >= c * Np) & (dst < (c + 1) * Np)
        e_ids = np.nonzero(m)[0]
        dst_c = dst[e_ids] - c * Np
        is_a = src[e_ids] < HALF
        # sort edges by (dst, group) so each node's A-edges precede B-edges
        order_e = np.lexsort((~is_a, dst_c))
        e_ids = e_ids[order_e]
        dst_c = dst_c[order_e]
        deg = np.bincount(dst_c, minlength=Np).astype(np.int64)
        degA = np.bincount(dst_c[src[e_ids] < HALF], minlength=Np).astype(np.int64)
        cum = np.zeros(Np + 1, dtype=np.int64)
        np.cumsum(deg, out=cum[1:])
        node_order = np.lexsort((-degA, -deg))
        pad_nodes = n_tiles * P - Np
        node_order_p = np.concatenate(
            [node_order, np.full(pad_nodes, -1, dtype=np.int64)])
        KAs, KBs = [], []
        for t in range(n_tiles):
            nt = node_order_p[t * P:(t + 1) * P]
            real = nt[nt >= 0]
            if len(real):
                KAs.append(int(degA[real].max()))
                KBs.append(int((deg[real] - degA[real]).max()))
            else:
                KAs.append(0)
                KBs.append(0)
        cores.append(dict(e_ids=e_ids, dst_c=dst_c, deg=deg, degA=degA,
                          cum=cum, node_order=node_order_p, KAs=KAs, KBs=KBs))

    # CA = self col + group-A slots (padded to 4); CB = group-B slots
    CAs, CBs = [], []
    for t in range(n_tiles):
        ka = max(c["KAs"][t] for c in cores)
        kb = max(c["KBs"][t] for c in cores)
        CAs.append(((1 + ka + 3) // 4) * 4)
        CBs.append(max(((kb + 3) // 4) * 4, 4))
    CAarr = np.array(CAs, dtype=np.int64)
    offsA = np.zeros(n_tiles + 1, dtype=np.int64)
    np.cumsum((CAarr - 1) * P, out=offsA[1:])       # gathered A slots
    offsB = np.zeros(n_tiles + 1, dtype=np.int64)
    np.cumsum(np.array(CBs, dtype=np.int64) * P, out=offsB[1:])
    offs4 = np.zeros(n_tiles + 1, dtype=np.int64)   # eaT4 quad-column offsets
    np.cumsum((CAarr + np.array(CBs)) // 4 * P, out=offs4[1:])
    SA, SB = int(offsA[-1]), int(offsB[-1])

    for c in range(NC):
        st = cores[c]
        deg, degA, cum = st["deg"], st["degA"], st["cum"]
        node_order = st["node_order"]
        idxA = np.zeros(SA, dtype=np.int16)   # 0 -> dummy-A row (j-order)
        idxB = np.zeros(SB, dtype=np.int16)   # 0 -> dummy-B row
        eaT4 = np.zeros((4 * ED, int(offs4[-1])), dtype=np.float32)
        rdeg = np.ones(n_tiles * P, dtype=np.float32)
        tile_of_pos = np.repeat(np.arange(n_tiles), P)
        p_of_pos = np.tile(np.arange(P), n_tiles)
        real_m = node_order >= 0
        nodes = node_order[real_m]
        rdeg[real_m] = 1.0 / np.maximum(deg[nodes], 1).astype(np.float32)
        pos_r = np.nonzero(real_m)[0]
        pos_of_node = np.empty(Np, dtype=np.int64)
        pos_of_node[nodes] = pos_r
        nloc = st["dst_c"]
        e_pos = pos_of_node[nloc]
        e_t = tile_of_pos[e_pos]
        e_p = p_of_pos[e_pos]
        r_in_node = np.arange(len(nloc)) - cum[nloc]   # 0..deg-1, A first
        e_srcs = src[st["e_ids"]]
        in_a = e_srcs < HALF
        rA = r_in_node
        rB = r_in_node - degA[nloc]
        jA = offsA[e_t[in_a]] + rA[in_a] * P + e_p[in_a]
        idxA[jA] = (e_srcs[in_a] + 1).astype(np.int16)
        jB = offsB[e_t[~in_a]] + rB[~in_a] * P + e_p[~in_a]
        idxB[jB] = (e_srcs[~in_a] - HALF + 1).astype(np.int16)

        # dma_gather SBUF index layout: value j at [j%16, j//16], the 16-row
        # block replicated 8x down the partitions (one copy per Q7 core pair)
        def _pack16(flat):
            cols = len(flat) // 16
            out2 = np.zeros((P, max(cols, 1)), dtype=np.int16)
            if cols:
                out2[:] = np.tile(flat.reshape(-1, 16).T, (8, 1))
            return out2
        # eaT4: group A edge -> in-group col 1+rA; group B edge -> col rB;
        # B quad block follows A quad block within each tile
        kg = np.where(in_a, 1 + rA, rB)
        qoff = np.where(in_a, 0, CAarr[e_t] // 4)
        col = offs4[e_t] + (qoff + (kg >> 2)) * P + e_p
        jj = (kg & 3).astype(np.int64)
        ea_c = ea[st["e_ids"]]
        for j4 in range(4):
            mj = jj == j4
            eaT4[j4 * ED:(j4 + 1) * ED, col[mj]] = ea_c[mj].T
        st["in"] = dict(idxA=_pack16(idxA), idxB=_pack16(idxB),
                        eaT4=eaT4, rdeg=rdeg)
    return cores, dict(CAs=CAs, CBs=CBs, offs4=offs4,
                       offsA=offsA, offsB=offsB, HALF=HALF)


# ---------------------------------------------------------------- device
def _build(cfg, meta, finalize=True):
    import concourse.bass as bass
    import concourse.bacc as bacc
    import concourse.tile as tile
    from concourse import mybir

    N, F, H, ED, NC = cfg["N"], cfg["F"], cfg["H"], cfg["ED"], cfg["NC"]
    Np = N // NC
    CAs, CBs = meta["CAs"], meta["CBs"]
    offs4 = meta["offs4"]
    offsA, offsB = meta["offsA"], meta["offsB"]
    HALF = meta["HALF"]
    n_tiles = len(CAs)
    FX = 192                 # table row width (f32): 768B, %256 for dma_gather
    FU = F + 2 * H           # used columns
    SA, SB = int(offsA[-1]), int(offsB[-1])
    f32 = mybir.dt.float32
    i16 = mybir.dt.int16
    AF = mybir.ActivationFunctionType
    OP = mybir.AluOpType
    EPS_IN, NEG = 1e-5, 0.2

    nc = bacc.Bacc("TRN2", target_bir_lowering=False, debug=False,
                   num_devices=NC)
    xT_d = nc.declare_dram_parameter("xT", [F, N], f32, isOutput=False)
    xTo_d = nc.declare_dram_parameter("xTo", [F, n_tiles * P], f32,
                                      isOutput=False)
    xo_d = nc.declare_dram_parameter("xo", [n_tiles * P, F], f32,
                                     isOutput=False)
    Wb_d = nc.declare_dram_parameter("Wb", [F, FX], f32, isOutput=False)
    v4_d = nc.declare_dram_parameter("v4", [4 * ED, 4 * H], f32, isOutput=False)
    ixA_d = nc.declare_dram_parameter("idxA", [P, max(SA // 16, 1)], i16,
                                      isOutput=False)
    ixB_d = nc.declare_dram_parameter("idxB", [P, max(SB // 16, 1)], i16,
                                      isOutput=False)
    ea4_d = nc.declare_dram_parameter("eaT4", [4 * ED, int(offs4[-1])], f32,
                                      isOutput=False)
    rdeg_d = nc.declare_dram_parameter("rdeg", [n_tiles * P], f32,
                                       isOutput=False)
    gam_d = nc.declare_dram_parameter("gamma", [F], f32, isOutput=False)
    bet_d = nc.declare_dram_parameter("beta", [F], f32, isOutput=False)
    out_d = nc.declare_dram_parameter("out", [n_tiles * P, F], f32,
                                      isOutput=True)

    with tile.TileContext(nc) as tc:
        with (
            tc.tile_pool(name="dram", bufs=1, space="DRAM") as dram,
            tc.tile_pool(name="consts", bufs=1) as consts,
            tc.tile_pool(name="ph_a", bufs=3) as pha,
            tc.tile_pool(name="ph_a_ps", bufs=2, space="PSUM") as pha_ps,
            tc.tile_pool(name="ph_b", bufs=2) as phb,
            tc.tile_pool(name="ph_b_ps", bufs=2, space="PSUM") as phb_ps,
            tc.tile_pool(name="stats_ps", bufs=2, space="PSUM") as stats_ps,
            tc.tile_pool(name="keep", bufs=1) as keep,
        ):
            hx = dram.tile([N + 2, FX], f32)

            Wb_s = consts.tile([F, FX], f32)
            nc.sync.dma_start(out=Wb_s[:], in_=Wb_d[:, :])
            v4_s = consts.tile([4 * ED, 4 * H], f32)
            nc.sync.dma_start(out=v4_s[:], in_=v4_d[:, :])
            ones = consts.tile([P, 1], f32)
            nc.vector.memset(ones[:], 1.0)

            # ---------------- Phase A: hx = x @ Wb  (full table, per core)
            # table rows: 0 dummy-A | 1..HALF nodes 0..HALF-1 |
            #             HALF+1 dummy-B | HALF+2.. nodes HALF..N-1
            n_chunks = math.ceil(N / P)
            for i in range(n_chunks):
                r0 = i * P
                nrow = min(P, N - r0)
                trow = r0 + 1 if r0 < HALF else r0 + 2
                xT_t = pha.tile([F, P], f32, name="xT_t")
                nc.sync.dma_start(out=xT_t[:, :nrow], in_=xT_d[:, r0:r0 + nrow])
                hx_p = pha_ps.tile([P, FX], f32, name="hx_p")
                nc.tensor.matmul(out=hx_p[:], lhsT=xT_t[:], rhs=Wb_s[:],
                                 start=True, stop=True)
                hx_s = pha.tile([P, FX], f32, name="hx_s")
                nc.vector.tensor_copy(out=hx_s[:], in_=hx_p[:])
                nc.sync.dma_start(out=hx[trow:trow + nrow, :], in_=hx_s[:nrow, :])
            dum = pha.tile([1, FX], f32, name="dum")
            nc.vector.memset(dum[:], 0.0)
            nc.vector.memset(dum[:, F:F + H], -1e30)
            nc.sync.dma_start(out=hx[0:1, :], in_=dum[:])
            nc.sync.dma_start(out=hx[HALF + 1:HALF + 2, :], in_=dum[:])

            # hx_own: own nodes in tile order (for self-loop slot + a_dst)
            hx_own = keep.tile([P, n_tiles, FU], f32)
            for t in range(n_tiles):
                xTo_t = pha.tile([F, P], f32, name="xTo_t")
                nc.sync.dma_start(out=xTo_t[:], in_=xTo_d[:, t * P:(t + 1) * P])
                ho_p = pha_ps.tile([P, FX], f32, name="ho_p", tag="hx_p")
                nc.tensor.matmul(out=ho_p[:], lhsT=xTo_t[:], rhs=Wb_s[:],
                                 start=True, stop=True)
                nc.vector.tensor_copy(out=hx_own[:, t, :], in_=ho_p[:, :FU])

            # ---------------- Phase B: per-tile attention + aggregation
            out_all = keep.tile([P, n_tiles, F], f32)
            acc = keep.tile([P, 2], f32)
            nc.vector.memset(acc[:], 0.0)

            GMAX = 8  # dma_gather caps out at ~1024 indices/instruction
            for t in range(n_tiles):
                CA, CB = CAs[t], CBs[t]
                den_acc = phb.tile([P, H], f32, name="den_acc", tag="den_acc")
                msg_acc = phb.tile([P, F], f32, name="msg_acc", tag="msg_acc")
                aeL_B = phb.tile([P, H], f32, name="aeL_B", tag="aeL_B")
                rdeg_t = phb.tile([P, 1], f32, name="rdeg_t", tag="rdeg_t")
                nc.sync.dma_start(out=rdeg_t[:],
                                  in_=rdeg_d[t * P:(t + 1) * P, None])

                # two passes: group B first (accumulators init), then group A
                # (self-loop col 0, needs aeL_B for the self a_edge)
                for is_a in (False, True):
                    C = CA if is_a else CB
                    C4 = C // 4
                    g = phb.tile([P, C, FX], f32, name="g", tag="g")
                    if is_a:
                        nc.vector.tensor_copy(out=g[:, 0, :FU],
                                              in_=hx_own[:, t, :])
                        ng = (C - 1) * P
                        o0, o1 = int(offsA[t]) // 16, int(offsA[t + 1]) // 16
                        ix_t = phb.tile([P, max(ng // 16, 1)], i16,
                                        name="ix_t", tag="ix_t")
                        if ng:
                            nc.sync.dma_start(out=ix_t[:, :],
                                              in_=ixA_d[:, o0:o1])
                        src_ap = hx[:, :]
                        gc0 = 1
                        q0 = 0
                    else:
                        ng = C * P
                        o0, o1 = int(offsB[t]) // 16, int(offsB[t + 1]) // 16
                        ix_t = phb.tile([P, max(ng // 16, 1)], i16,
                                        name="ix_t", tag="ix_t")
                        if ng:
                            nc.sync.dma_start(out=ix_t[:, :],
                                              in_=ixB_d[:, o0:o1])
                        src_ap = hx[HALF + 1:, :]
                        gc0 = 0
                        q0 = CA // 4
                    ncols = ng // P
                    for g0 in range(0, ncols, GMAX):
                        kk = min(GMAX, ncols - g0)
                        nc.gpsimd.dma_gather(
                            out_ap=g[:, gc0 + g0:gc0 + g0 + kk, :],
                            in_ap=src_ap,
                            idxs_ap=ix_t[:, g0 * 8:(g0 + kk) * 8],
                            num_idxs=kk * P,
                            num_idxs_reg=kk * P,
                            elem_size=FX,
                        )
                    ea4_t = phb.tile([4 * ED, C4 * P], f32, name="ea4_t",
                                     tag="ea4_t")
                    nc.sync.dma_start(
                        out=ea4_t[:],
                        in_=ea4_d[:, int(offs4[t]) + q0 * P:
                                  int(offs4[t]) + (q0 + C4) * P])

                    # a_edge: quad matmuls [4ED,P] @ [4ED,4H]
                    ae = phb.tile([P, C, H], f32, name="ae", tag="ae")
                    QG = 16
                    for qg in range(math.ceil(C4 / QG)):
                        nq = min(QG, C4 - qg * QG)
                        ae_p = phb_ps.tile([P, QG * 4 * H], f32, name="ae_p",
                                           tag="ae_p")
                        for qi in range(nq):
                            q = qg * QG + qi
                            nc.tensor.matmul(
                                out=ae_p[:, qi * 4 * H:(qi + 1) * 4 * H],
                                lhsT=ea4_t[:, q * P:(q + 1) * P],
                                rhs=v4_s[:],
                                start=True, stop=True)
                        nc.vector.tensor_copy(
                            out=ae[:, qg * QG * 4:qg * QG * 4 + nq * 4, :],
                            in_=ae_p[:, :nq * 4 * H])
                    aeL = phb.tile([P, H], f32, name="aeL", tag="aeL")
                    nc.vector.tensor_reduce(
                        out=aeL[:], in_=ae.transpose([0, 2, 1]),
                        axis=mybir.AxisListType.X, op=OP.add)
                    if not is_a:
                        nc.vector.tensor_copy(out=aeL_B[:], in_=aeL[:])
                    else:
                        # self-loop a_edge = (sum of a_edge over ALL slots)/deg
                        nc.vector.tensor_add(aeL[:], aeL[:], aeL_B[:])
                        nc.vector.tensor_scalar_mul(ae[:, 0, :], aeL[:],
                                                    rdeg_t[:])

                    # logits -> exp(leaky) ; no segment max (clamped at -88)
                    al = phb.tile([P, H, C], f32, name="al", tag="al")
                    alv = al.transpose([0, 2, 1])
                    nc.vector.tensor_tensor(
                        out=alv, in0=g[:, :, F:F + H], in1=ae[:, :, :],
                        op=OP.add)
                    adst = hx_own[:, t, F + H:F + 2 * H]
                    nc.vector.tensor_tensor(
                        out=alv, in0=alv,
                        in1=adst.unsqueeze(1).broadcast_to((P, C, H)),
                        op=OP.add)
                    tl = phb.tile([P, H, C], f32, name="tl", tag="tl")
                    nc.vector.tensor_scalar_mul(tl[:], al[:], NEG)
                    nc.vector.tensor_tensor(out=al[:], in0=al[:], in1=tl[:],
                                            op=OP.max)
                    nc.vector.tensor_scalar_max(al[:], al[:], -88.0)
                    nc.scalar.activation(out=al[:], in_=al[:], func=AF.Exp)
                    # accumulate denominator and weighted messages
                    if not is_a:
                        nc.vector.tensor_reduce(
                            out=den_acc[:], in_=al[:],
                            axis=mybir.AxisListType.X, op=OP.add)
                    else:
                        den_t = phb.tile([P, H], f32, name="den_t",
                                         tag="den_t")
                        nc.vector.tensor_reduce(
                            out=den_t[:], in_=al[:],
                            axis=mybir.AxisListType.X, op=OP.add)
                        nc.vector.tensor_add(den_acc[:], den_acc[:], den_t[:])
                    gh = g[:, :, 0:F].rearrange("p k (h d) -> p k h d", h=H)
                    nc.vector.tensor_tensor(
                        out=gh, in0=gh,
                        in1=al.transpose([0, 2, 1]).unsqueeze(3)
                            .broadcast_to((P, C, H, F // H)),
                        op=OP.mult)
                    if not is_a:
                        nc.vector.tensor_reduce(
                            out=msg_acc[:],
                            in_=g[:, :, 0:F].transpose([0, 2, 1]),
                            axis=mybir.AxisListType.X, op=OP.add)
                    else:
                        msg_t = phb.tile([P, F], f32, name="msg_t",
                                         tag="msg_t")
                        nc.vector.tensor_reduce(
                            out=msg_t[:],
                            in_=g[:, :, 0:F].transpose([0, 2, 1]),
                            axis=mybir.AxisListType.X, op=OP.add)
                        nc.vector.tensor_add(msg_acc[:], msg_acc[:], msg_t[:])

                # out_pre = msg / den  (per-node alpha normalization)
                rec = phb.tile([P, H], f32, name="rec", tag="rec")
                nc.vector.tensor_scalar_add(rec[:], den_acc[:], 1e-16)
                nc.vector.reciprocal(rec[:], rec[:])
                op_t = out_all[:, t, :]
                nc.vector.tensor_tensor(
                    out=op_t.rearrange("p (h d) -> p h d", h=H),
                    in0=msg_acc.rearrange("p (h d) -> p h d", h=H),
                    in1=rec.unsqueeze(2).broadcast_to((P, H, F // H)),
                    op=OP.mult)

                # stats: per-channel sum & sumsq via ones-matmul
                sq = phb.tile([P, F], f32, name="sq", tag="sq")
                nc.vector.tensor_mul(sq[:], op_t, op_t)
                st_p = stats_ps.tile([P, 2], f32, name="st_p", tag="st_p")
                nc.tensor.matmul(out=st_p[:, 0:1], lhsT=op_t, rhs=ones[:],
                                 start=True, stop=True)
                nc.tensor.matmul(out=st_p[:, 1:2], lhsT=sq[:], rhs=ones[:],
                                 start=True, stop=True)
                nc.vector.tensor_add(acc[:], acc[:], st_p[:])

            # ---------------- Phase C: stats allreduce + normalize + ELU
            st_in = dram.tile([P, 2], f32)
            st_out = dram.tile([P, 2], f32, addr_space="Shared")
            nc.sync.dma_start(out=st_in[:], in_=acc[:])
            nc.gpsimd.collective_compute(
                "AllReduce", OP.add,
                replica_groups=[list(range(NC))],
                ins=[st_in[:].opt()], outs=[st_out[:].opt()])
            sg = keep.tile([P, 2], f32)
            nc.sync.dma_start(out=sg[:], in_=st_out[:])
            mean = keep.tile([P, 1], f32)
            nc.vector.tensor_scalar_mul(mean[:], sg[:, 0:1], 1.0 / N)
            ex2 = keep.tile([P, 1], f32)
            nc.vector.tensor_scalar_mul(ex2[:], sg[:, 1:2], 1.0 / N)
            var = keep.tile([P, 1], f32)
            nc.vector.tensor_mul(var[:], mean[:], mean[:])
            nc.vector.tensor_sub(var[:], ex2[:], var[:])
            rstd = keep.tile([P, 1], f32)
            eps_t = keep.tile([P, 1], f32)
            nc.vector.memset(eps_t[:], EPS_IN)
            nc.scalar.activation(out=rstd[:], in_=var[:], func=AF.Sqrt,
                                 bias=eps_t[:])
            nc.vector.reciprocal(rstd[:], rstd[:])
            gam_s = keep.tile([P, 1], f32)
            nc.sync.dma_start(out=gam_s[:], in_=gam_d[:, None])
            bet_s = keep.tile([P, 1], f32)
            nc.sync.dma_start(out=bet_s[:], in_=bet_d[:, None])
            scl = keep.tile([P, 1], f32)
            nc.vector.tensor_mul(scl[:], rstd[:], gam_s[:])
            bia = keep.tile([P, 1], f32)
            nc.vector.tensor_mul(bia[:], mean[:], scl[:])
            nc.vector.tensor_sub(bia[:], bet_s[:], bia[:])
            sb_dram = dram.tile([2, P], f32)
            nc.sync.dma_start(out=sb_dram[0, :], in_=scl[:, 0])
            nc.sync.dma_start(out=sb_dram[1, :], in_=bia[:, 0])
            sclB = keep.tile([P, F], f32)
            nc.sync.dma_start(out=sclB[:],
                              in_=sb_dram[0:1, :].broadcast_to((P, P)))
            biaB = keep.tile([P, F], f32)
            nc.sync.dma_start(out=biaB[:],
                              in_=sb_dram[1:2, :].broadcast_to((P, P)))

            with tc.tile_pool(name="ph_c", bufs=3) as phc:
                for t in range(n_tiles):
                    xo_t = phc.tile([P, F], f32, name="xo_t")
                    nc.sync.dma_start(out=xo_t[:],
                                      in_=xo_d[t * P:(t + 1) * P, :])
                    z = phc.tile([P, F], f32, name="z")
                    nc.vector.tensor_mul(z[:], out_all[:, t, :], sclB[:])
                    nc.vector.tensor_add(z[:], z[:], biaB[:])
                    nc.vector.tensor_add(z[:], z[:], xo_t[:])
                    zm = phc.tile([P, F], f32, name="zm")
                    nc.vector.tensor_scalar_min(zm[:], z[:], 0.0)
                    nc.scalar.activation(out=zm[:], in_=zm[:], func=AF.Exp)
                    nc.vector.tensor_scalar_max(z[:], z[:], 0.0)
                    nc.vector.tensor_add(z[:], z[:], zm[:])
                    nc.vector.tensor_scalar_add(z[:], z[:], -1.0)
                    nc.sync.dma_start(out=out_d[t * P:(t + 1) * P, :], in_=z[:])
    if finalize:
        nc.finalize()
    return nc


# ---------------------------------------------------------------- driver
def _run_gat(x, edge_index, edge_attr, W, att_src, att_dst, W_e, att_edge,
             gamma, beta, cfg, trace=False, return_results=False):
    from concourse.bass_utils import run_bass_kernel_spmd

    N, F, H, Dh, NC = cfg["N"], cfg["F"], cfg["H"], cfg["Dh"], cfg["NC"]
    Np = N // NC
    FX = 192
    Wb, v4 = _fold_weights(
        np.asarray(W, np.float32), np.asarray(att_src, np.float32),
        np.asarray(att_dst, np.float32), np.asarray(W_e, np.float32),
        np.asarray(att_edge, np.float32), H, Dh, FX)
    cores, meta = _preprocess(edge_index, edge_attr, cfg)
    nc = _build(cfg, meta)

    x_np = np.asarray(x, np.float32)
    xT = np.ascontiguousarray(x_np.T)
    gam = np.asarray(gamma, np.float32)
    bet = np.asarray(beta, np.float32)
    n_tiles = len(meta["CAs"])
    in_maps = []
    for c in range(NC):
        st = cores[c]["in"]
        order = cores[c]["node_order"]
        gl = np.where(order >= 0, c * Np + order, 0)
        xo = x_np[gl]
        xo[order < 0] = 0.0
        xTo = np.ascontiguousarray(xo.T)
        in_maps.append(dict(
            xT=xT, xTo=xTo, xo=np.ascontiguousarray(xo), Wb=Wb, v4=v4,
            idxA=st["idxA"], idxB=st["idxB"],
            eaT4=st["eaT4"], rdeg=st["rdeg"], gamma=gam, beta=bet))
    res = run_bass_kernel_spmd(nc, in_maps, core_ids=list(range(NC)),
                               trace=trace)
    out = np.empty((N, F), dtype=np.float32)
    for c in range(NC):
        oc = res.results[c]["out"]
        order = cores[c]["node_order"]
        real = order >= 0
        out[c * Np + order[real]] = oc[np.nonzero(real)[0]]
    if return_results:
        return out, res
    return out


def kernel(x, edge_index, edge_attr, W, att_src, att_dst, W_e, att_edge,
           gamma, beta):
    return _run_gat(x, edge_index, edge_attr, W, att_src, att_dst, W_e,
                    att_edge, gamma, beta, _cfg_full())



# revision 2
# speedup vs baseline: 1.0018x; 1.0018x over previous
"""GAT block (GATConv + InstanceNorm + residual + ELU) on 8 Trainium2 cores.

v2 strategy (gather-ucode-minimal):
  - dst-node graph parallel across 8 cores; nodes snake-dealt to cores by
    global in-degree, then deg-sorted into 128-node tiles so per-tile max
    degree (= slot columns) is minimal and aligned across cores.
  - ONE gather index per edge: the DRAM table packs NODE PAIRS per row
    (768B: [h(2k) bf16 128 | asrc(2k) | pad | h(2k+1) | asrc(2k+1) | pad]),
    so idx = src>>1 fits int16 with no A/B table split. A host-built
    {0,-1e30} mask picks the even/odd half in the logits (wrong half's
    alpha underflows to 0).
  - per-tile slot layout [dst=128 partitions, slot cols, 384 bf16]; alpha
    written into the row's pad region so one halving-tree accumulates
    messages AND softmax denominators; all DVE ops bf16/contiguous.
  - a_edge via TensorE on host-packed 8-slot-interleaved eaT8 (bf16);
    self-loop edge_attr = mean of incoming, via linearity.
  - InstanceNorm stats via ones-matmul + AllReduce; finalize = affine +
    residual + ELU (fp32).
"""

import math
import numpy as np

P = 128
F, H, Dh, ED = 128, 8, 16, 16
FXE = 192          # bf16 elems per node block in a table row
ROWW = 2 * FXE     # pair row width (384 bf16 = 768B)
KCAP = 40          # max edge slot-cols per chunk
GMAX = 8           # slot-cols per gather instruction (1024 idxs)
EPS_IN, NEG, MNEG = 1e-5, 0.2, -1e30


def _cfg_full():
    return dict(N=50000, E=1600000, NC=8)


def _fold_weights(W, att_src, att_dst, W_e, att_edge):
    import ml_dtypes
    w_src = np.stack(
        [W[:, h * Dh:(h + 1) * Dh] @ att_src[h] for h in range(H)], axis=1)
    w_dst = np.stack(
        [W[:, h * Dh:(h + 1) * Dh] @ att_dst[h] for h in range(H)], axis=1)
    Wb = np.concatenate([W, w_src, w_dst], axis=1)  # [F, 144]
    v = np.stack(
        [W_e[:, h * Dh:(h + 1) * Dh] @ att_edge[h] for h in range(H)], axis=1)
    v8 = np.zeros((8 * ED, 8 * H), dtype=np.float32)
    for s in range(8):
        v8[s * ED:(s + 1) * ED, s * H:(s + 1) * H] = v
    return Wb.astype(ml_dtypes.bfloat16), v8.astype(ml_dtypes.bfloat16)


def _chunks_of(K):
    """Chunk list for a tile with K edge slots: [(j0, ne, has_self), ...].
    Chunk 0 (with the self col) is listed first; device processes it LAST."""
    ch = [(0, min(K, KCAP - 1), True)]
    j = KCAP - 1
    while j < K:
        ch.append((j, min(KCAP, K - j), False))
        j += KCAP
    return ch


def _pack16(flat):
    cols = len(flat) // 16
    out2 = np.zeros((P, max(cols, 1)), dtype=np.int16)
    if cols:
        out2[:] = np.tile(flat.reshape(-1, 16).T, (8, 1))
    return out2


def _preprocess(x, edge_index, edge_attr, cfg):
    import ml_dtypes
    N, E, NC = cfg["N"], cfg["E"], cfg["NC"]
    Np = N // NC
    n_tiles = math.ceil(Np / P)
    src = np.asarray(edge_index[0]).astype(np.int64)
    dst = np.asarray(edge_index[1]).astype(np.int64)
    ea = np.asarray(edge_attr, dtype=np.float32)
    x_np = np.asarray(x, dtype=np.float32)

    # ---- node -> (core, tile, partition): global-degree snake deal
    deg_g = np.bincount(dst, minlength=N)
    order = np.argsort(-deg_g, kind="stable")
    ranks = np.arange(N)
    blk, pos = ranks // NC, ranks % NC
    core_of_rank = np.where(blk % 2 == 0, pos, NC - 1 - pos)
    assign = np.empty(N, dtype=np.int64)
    assign[order] = core_of_rank
    local_rank = np.empty(N, dtype=np.int64)
    nodes_of_core = []
    for c in range(NC):
        nodes_c = order[core_of_rank == c]          # deg-desc order
        assert len(nodes_c) == Np
        local_rank[nodes_c] = np.arange(Np)
        nodes_of_core.append(nodes_c)

    # ---- per-core edge routing and per-tile max degree
    cores = []
    Kct = np.zeros((NC, n_tiles), dtype=np.int64)
    for c in range(NC):
        m = assign[dst] == c
        e_ids = np.nonzero(m)[0]
        dl = local_rank[dst[e_ids]]
        o = np.argsort(dl, kind="stable")
        e_ids, dl = e_ids[o], dl[o]
        deg = np.bincount(dl, minlength=Np)
        cum = np.zeros(Np + 1, dtype=np.int64)
        np.cumsum(deg, out=cum[1:])
        j_e = np.arange(len(dl)) - cum[dl]
        t_e, p_e = dl // P, dl % P
        np.maximum.at(Kct[c], t_e, j_e + 1)
        cores.append(dict(e_ids=e_ids, dl=dl, j=j_e, t=t_e, p=p_e, deg=deg))

    K_t = Kct.max(axis=0)

    # ---- shared chunk schedule + offsets (identical across cores)
    chunks = []          # (t, j0, ne, has_self, C, EQ)
    for t in range(n_tiles):
        for (j0, ne, hs) in _chunks_of(int(K_t[t])):
            C = ne + (1 if hs else 0)
            EQ = (ne + 7) // 8
            chunks.append((t, j0, ne, hs, C, EQ))
    n_chunks = len(chunks)
    idx_off = np.zeros(n_chunks + 1, dtype=np.int64)   # in idxs
    mask_off = np.zeros(n_chunks + 1, dtype=np.int64)  # in cols (per partition)
    ea_off = np.zeros(n_chunks + 1, dtype=np.int64)    # in cols
    for i, (t, j0, ne, hs, C, EQ) in enumerate(chunks):
        idx_off[i + 1] = idx_off[i] + ne * P
        mask_off[i + 1] = mask_off[i] + C * 16
        ea_off[i + 1] = ea_off[i] + EQ * P
    chunk_no_of = {}
    for i, (t, j0, ne, hs, C, EQ) in enumerate(chunks):
        chunk_no_of[(t, j0)] = i

    # chunk id lookup for an edge slot j: piecewise
    def _ci_arrays(j):
        in0 = j < (KCAP - 1)
        ci = np.where(in0, 0, 1 + (j - (KCAP - 1)) // KCAP)
        j0 = np.where(in0, 0, (KCAP - 1) + ((j - (KCAP - 1)) // KCAP) * KCAP)
        jj = j - j0
        cc = jj + np.where(in0, 1, 0)   # col within chunk (self col shifts)
        return ci, j0, jj, cc

    ea_bf = ea.astype(ml_dtypes.bfloat16)
    SIDX = int(idx_off[-1])
    SMASK = int(mask_off[-1])
    SEA = int(ea_off[-1])

    # tile-major base chunk numbers
    for c in range(NC):
        st = cores[c]
        t_e, p_e, j_e = st["t"], st["p"], st["j"]
        src_e = src[st["e_ids"]]
        ci, j0, jj, cc = _ci_arrays(j_e)
        # vectorized chunk_no: build lookup [n_tiles, max_ci]
        max_ci = 1 + max(0, (int(K_t.max()) - (KCAP - 1) + KCAP - 1) // KCAP)
        lut = np.full((n_tiles, max_ci + 1), -1, dtype=np.int64)
        for i, (t, jj0, ne, hs, C, EQ) in enumerate(chunks):
            cidx = 0 if hs else 1 + (jj0 - (KCAP - 1)) // KCAP
            lut[t, cidx] = i
        cno = lut[t_e, ci]
        assert (cno >= 0).all()

        idxA = np.zeros(SIDX, dtype=np.int16)
        idxA[idx_off[cno] + jj * P + p_e] = (src_e >> 1).astype(np.int16)
        maskA = np.full((P, SMASK), MNEG, dtype=ml_dtypes.bfloat16)
        colm = (mask_off[cno] + cc * 16 + (src_e & 1) * 8).astype(np.int64)
        maskA[p_e[:, None], colm[:, None] + np.arange(8)[None, :]] = 0.0
        # self cols: even half active
        for i, (t, jj0, ne, hs, C, EQ) in enumerate(chunks):
            if hs:
                maskA[:, int(mask_off[i]):int(mask_off[i]) + 8] = 0.0
        ea8 = np.zeros((8 * ED, SEA), dtype=ml_dtypes.bfloat16)
        q_e, s_e = jj // 8, jj % 8
        cole = (ea_off[cno] + q_e * P + p_e).astype(np.int64)
        rows = (s_e[:, None] * ED + np.arange(ED)[None, :]).astype(np.int64)
        ea8[rows, cole[:, None]] = ea_bf[st["e_ids"]]

        rdeg = np.ones((P, n_tiles), dtype=np.float32)
        deg = st["deg"]
        idxs = np.arange(Np)
        rdeg[idxs % P, idxs // P] = 1.0 / np.maximum(deg, 1.0)

        nodes_c = nodes_of_core[c]
        pad = n_tiles * P - Np
        xo = np.zeros((n_tiles * P, F), dtype=np.float32)
        xo[:Np] = x_np[nodes_c]
        xTo = np.ascontiguousarray(xo.T).astype(ml_dtypes.bfloat16)
        st["in"] = dict(idx=_pack16(idxA), mask=maskA, ea8=ea8, rdeg=rdeg,
                        xo=xo, xTo=xTo)

    # pair-interleaved xT for Phase A (shared by all cores); evens at
    # partitions 0..63, odds at 64..127 of each 128-node chunk, zero-padded
    n_chunksA = math.ceil(N / P)
    xpad = np.zeros((n_chunksA * P, F), dtype=np.float32)
    for i0 in range(0, N, P):
        nrow = min(P, N - i0)
        assert nrow % 2 == 0
        xpad[i0:i0 + nrow // 2] = x_np[i0:i0 + nrow:2]
        xpad[i0 + 64:i0 + 64 + nrow // 2] = x_np[i0 + 1:i0 + nrow:2]
    xT_pa = np.ascontiguousarray(xpad.T).astype(ml_dtypes.bfloat16)

    meta = dict(N=N, NC=NC, Np=Np, n_tiles=n_tiles, K_t=K_t, chunks=chunks,
                idx_off=idx_off, mask_off=mask_off, ea_off=ea_off,
                SIDX=SIDX, SMASK=SMASK, SEA=SEA)
    return cores, nodes_of_core, xT_pa, meta


# ---------------------------------------------------------------- device
def _build(meta, finalize=True):
    import concourse.bass as bass
    import concourse.bacc as bacc
    import concourse.tile as tile
    from concourse import mybir

    N, NC = meta["N"], meta["NC"]
    n_tiles = meta["n_tiles"]
    chunks = meta["chunks"]
    idx_off, mask_off, ea_off = meta["idx_off"], meta["mask_off"], meta["ea_off"]
    SIDX, SMASK, SEA = meta["SIDX"], meta["SMASK"], meta["SEA"]
    NPAIR = N // 2
    f32 = mybir.dt.float32
    bf16 = mybir.dt.bfloat16
    i16 = mybir.dt.int16
    AF = mybir.ActivationFunctionType
    OP = mybir.AluOpType
    KMAX = int(max(c[4] for c in chunks))   # max C

    n_chunksA = math.ceil(N / P)
    nc = bacc.Bacc("TRN2", target_bir_lowering=False, debug=False,
                   num_devices=NC)
    xT_d = nc.declare_dram_parameter("xT", [F, n_chunksA * P], bf16,
                                     isOutput=False)
    xTo_d = nc.declare_dram_parameter("xTo", [F, n_tiles * P], bf16,
                                      isOutput=False)
    xo_d = nc.declare_dram_parameter("xo", [n_tiles * P, F], f32,
                                     isOutput=False)
    Wb_d = nc.declare_dram_parameter("Wb", [F, 144], bf16, isOutput=False)
    v8_d = nc.declare_dram_parameter("v8", [8 * ED, 8 * H], bf16,
                                     isOutput=False)
    ix_d = nc.declare_dram_parameter("idx", [P, max(SIDX // 16, 1)], i16,
                                     isOutput=False)
    mk_d = nc.declare_dram_parameter("mask", [P, SMASK], bf16, isOutput=False)
    ea_d = nc.declare_dram_parameter("ea8", [8 * ED, SEA], bf16,
                                     isOutput=False)
    rdeg_d = nc.declare_dram_parameter("rdeg", [P, n_tiles], f32,
                                       isOutput=False)
    gam_d = nc.declare_dram_parameter("gamma", [F], f32, isOutput=False)
    bet_d = nc.declare_dram_parameter("beta", [F], f32, isOutput=False)
    out_d = nc.declare_dram_parameter("out", [n_tiles * P, F], f32,
                                      isOutput=True)

    with tile.TileContext(nc) as tc:
        with (
            tc.tile_pool(name="dram", bufs=1, space="DRAM") as dram,
            tc.tile_pool(name="consts", bufs=1) as consts,
            tc.tile_pool(name="keep", bufs=1) as keep,
        ):
            hx = dram.tile([NPAIR, ROWW], bf16)

            Wb_s = consts.tile([F, 144], bf16)
            nc.sync.dma_start(out=Wb_s[:], in_=Wb_d[:, :])
            v8_s = consts.tile([8 * ED, 8 * H], bf16)
            nc.sync.dma_start(out=v8_s[:], in_=v8_d[:, :])
            ones = consts.tile([P, 1], f32)
            nc.vector.memset(ones[:], 1.0)
            rdeg_s = consts.tile([P, n_tiles], f32)
            nc.sync.dma_start(out=rdeg_s[:], in_=rdeg_d[:, :])

            hx_own = keep.tile([P, n_tiles, 144], bf16)
            out_all = keep.tile([P, n_tiles, F], f32)
            acc = keep.tile([P, 2], f32)
            nc.vector.memset(acc[:], 0.0)

            # ---------------- Phase A: pair table hx = x @ Wb
            # 8 node-chunks share one staging tile -> 2 bulk DMAs per group
            with (
                tc.tile_pool(name="pha", bufs=4) as pha,
                tc.tile_pool(name="pha_ps", bufs=4, space="PSUM") as pha_ps,
            ):
                CB = 8
                n_full = N // P           # full 128-node chunks
                for i0 in range(0, n_chunksA, CB):
                    nb = min(CB, n_chunksA - i0)
                    bulk = (i0 + nb <= n_full)   # all chunks full-size
                    st8 = pha.tile([P, CB, FXE], bf16, name="st8", tag="st8")
                    nc.vector.memset(st8[:, :, 136:FXE], 0.0)
                    for j in range(0, nb, 4):
                        nx = min(4, nb - j)
                        xT_t = pha.tile([F, 4 * P], bf16, name="xT_t",
                                        tag="xT_t")
                        nc.scalar.dma_start(
                            out=xT_t[:, 0:nx * P],
                            in_=xT_d[:, (i0 + j) * P:(i0 + j + nx) * P])
                        for k in range(nx):
                            hp = pha_ps.tile([P, 144], f32, name="hp",
                                             tag="hp")
                            nc.tensor.matmul(out=hp[:],
                                             lhsT=xT_t[:, k * P:(k + 1) * P],
                                             rhs=Wb_s[:],
                                             start=True, stop=True)
                            nc.vector.tensor_copy(
                                out=st8[:, j + k, 0:136], in_=hp[:, 0:136])
                    prg = i0 * 64
                    if bulk:
                        nc.sync.dma_start(
                            out=hx[prg:prg + nb * 64, 0:FXE]
                                .rearrange("(c p) f -> p c f", p=64),
                            in_=st8[0:64, 0:nb, :])
                        nc.scalar.dma_start(
                            out=hx[prg:prg + nb * 64, FXE:ROWW]
                                .rearrange("(c p) f -> p c f", p=64),
                            in_=st8[64:128, 0:nb, :])
                    else:
                        for j in range(nb):
                            r0 = (i0 + j) * P
                            npair = min(P, N - r0) // 2
                            pr0 = r0 // 2
                            nc.sync.dma_start(
                                out=hx[pr0:pr0 + npair, 0:FXE],
                                in_=st8[0:npair, j, :])
                            nc.scalar.dma_start(
                                out=hx[pr0:pr0 + npair, FXE:ROWW],
                                in_=st8[64:64 + npair, j, :])
                # own nodes (tile order): h | asrc | adst, fp32
                for t in range(n_tiles):
                    xTo_t = pha.tile([F, P], bf16, name="xTo_t", tag="xT_t")
                    nc.sync.dma_start(out=xTo_t[:],
                                      in_=xTo_d[:, t * P:(t + 1) * P])
                    hp = pha_ps.tile([P, 144], f32, name="hp2", tag="hp")
                    nc.tensor.matmul(out=hp[:], lhsT=xTo_t[:], rhs=Wb_s[:],
                                     start=True, stop=True)
                    nc.vector.tensor_copy(out=hx_own[:, t, :], in_=hp[:])

            # ---------------- Phase B: per-tile attention + aggregation
            with (
                tc.tile_pool(name="phb", bufs=3) as phb,
                tc.tile_pool(name="acc_p", bufs=2) as accp,
                tc.tile_pool(name="phb_ps", bufs=4, space="PSUM") as phb_ps,
                tc.tile_pool(name="st_ps", bufs=2, space="PSUM") as st_ps,
            ):
                for t in range(n_tiles):
                    tile_chunks = [ch for ch in chunks if ch[0] == t]
                    # process non-self chunks first, self chunk last
                    tile_chunks = ([c for c in tile_chunks if not c[3]]
                                   + [c for c in tile_chunks if c[3]])
                    msg_acc = accp.tile([P, F], f32, name="msg_acc",
                                        tag="msg_acc")
                    den_acc = accp.tile([P, H], f32, name="den_acc",
                                        tag="den_acc")
                    aeL_acc = accp.tile([P, H], f32, name="aeL_acc",
                                        tag="aeL_acc")
                    nc.vector.memset(msg_acc[:], 0.0)
                    nc.vector.memset(den_acc[:], 0.0)
                    nc.vector.memset(aeL_acc[:], 0.0)
                    for (tt, j0, ne, hs, C, EQ) in tile_chunks:
                        cno = None
                        for i, ch in enumerate(chunks):
                            if ch[0] == t and ch[1] == j0:
                                cno = i
                                break
                        e0 = 1 if hs else 0
                        g = phb.tile([P, KMAX, ROWW], bf16, name="g", tag="g")
                        mk = phb.tile([P, KMAX, 16], bf16, name="mk", tag="mk")
                        nc.scalar.dma_start(
                            out=mk[:, 0:C, :].rearrange("p c h -> p (c h)"),
                            in_=mk_d[:, int(mask_off[cno]):int(mask_off[cno + 1])])
                        if ne:
                            ixt = phb.tile([P, KCAP * 8], i16, name="ixt",
                                           tag="ixt")
                            o0 = int(idx_off[cno]) // 16
                            nc.scalar.dma_start(out=ixt[:, 0:ne * 8],
                                                in_=ix_d[:, o0:o0 + ne * 8])
                            ea8 = phb.tile([8 * ED, (KCAP // 8) * P], bf16,
                                           name="ea8", tag="ea8")
                            nc.sync.dma_start(
                                out=ea8[:, 0:EQ * P],
                                in_=ea_d[:, int(ea_off[cno]):int(ea_off[cno + 1])])
                        if hs:
                            nc.vector.tensor_copy(out=g[:, 0, 0:136],
                                                  in_=hx_own[:, t, 0:136])
                            nc.vector.tensor_copy(out=g[:, 0, FXE:FXE + 136],
                                                  in_=hx_own[:, t, 0:136])
                        for g0 in range(0, ne, GMAX):
                            kk = min(GMAX, ne - g0)
                            nc.gpsimd.dma_gather(
                                out_ap=g[:, e0 + g0:e0 + g0 + kk, :],
                                in_ap=hx[:, :],
                                idxs_ap=ixt[:, g0 * 8:(g0 + kk) * 8],
                                num_idxs=kk * P,
                                num_idxs_reg=kk * P,
                                elem_size=ROWW,
                            )
                        # a_edge
                        ae = phb.tile([P, KMAX, H], f32, name="ae", tag="ae")
                        for q in range(EQ):
                            aep = phb_ps.tile([P, 8 * H], f32, name="aep",
                                              tag="aep")
                            nc.tensor.matmul(
                                out=aep[:],
                                lhsT=ea8[:, q * P:(q + 1) * P],
                                rhs=v8_s[:], start=True, stop=True)
                            nq = min(8, ne - q * 8)
                            nc.vector.tensor_copy(
                                out=ae[:, e0 + q * 8:e0 + q * 8 + nq, :]
                                    .rearrange("p c h -> p (c h)"),
                                in_=aep[:, 0:nq * H])
                        if ne:
                            aeL = phb.tile([P, H], f32, name="aeL", tag="aeL")
                            nc.vector.tensor_reduce(
                                out=aeL[:],
                                in_=ae[:, e0:e0 + ne, :].transpose([0, 2, 1]),
                                axis=mybir.AxisListType.X, op=OP.add)
                            nc.vector.tensor_add(aeL_acc[:], aeL_acc[:],
                                                 aeL[:])
                        if hs:
                            nc.vector.tensor_scalar_mul(
                                ae[:, 0, :], aeL_acc[:], rdeg_s[:, t:t + 1])
                        # logits [P, C, 16] fp32
                        al = phb.tile([P, KMAX, 16], f32, name="al", tag="al")
                        adst_b = hx_own[:, t, 136:144].unsqueeze(1) \
                            .broadcast_to((P, C, H))
                        nc.vector.tensor_tensor(
                            out=al[:, 0:C, 0:8], in0=ae[:, 0:C, :],
                            in1=adst_b, op=OP.add)
                        nc.vector.tensor_tensor(
                            out=al[:, 0:C, 8:16], in0=ae[:, 0:C, :],
                            in1=adst_b, op=OP.add)
                        nc.vector.tensor_tensor(
                            out=al[:, 0:C, 0:8], in0=al[:, 0:C, 0:8],
                            in1=g[:, 0:C, 128:136], op=OP.add)
                        nc.vector.tensor_tensor(
                            out=al[:, 0:C, 8:16], in0=al[:, 0:C, 8:16],
                            in1=g[:, 0:C, FXE + 128:FXE + 136], op=OP.add)
                        nc.vector.tensor_tensor(
                            out=al[:, 0:C, :], in0=al[:, 0:C, :],
                            in1=mk[:, 0:C, :], op=OP.add)
                        nc.vector.scalar_tensor_tensor(
                            out=al[:, 0:C, :], in0=al[:, 0:C, :], scalar=NEG,
                            in1=al[:, 0:C, :], op0=OP.mult, op1=OP.max)
                        nc.vector.tensor_scalar_max(al[:, 0:C, :],
                                                    al[:, 0:C, :], -88.0)
                        nc.scalar.activation(out=g[:, 0:C, 136:152],
                                             in_=al[:, 0:C, :], func=AF.Exp)
                        # fold alpha into h (both halves)
                        nc.vector.tensor_tensor(
                            out=g[:, 0:C, 0:128].rearrange(
                                "p c (h d) -> p c h d", h=H),
                            in0=g[:, 0:C, 0:128].rearrange(
                                "p c (h d) -> p c h d", h=H),
                            in1=g[:, 0:C, 136:144].unsqueeze(3)
                                .broadcast_to((P, C, H, Dh)),
                            op=OP.mult)
                        nc.vector.tensor_tensor(
                            out=g[:, 0:C, FXE:FXE + 128].rearrange(
                                "p c (h d) -> p c h d", h=H),
                            in0=g[:, 0:C, FXE:FXE + 128].rearrange(
                                "p c (h d) -> p c h d", h=H),
                            in1=g[:, 0:C, 144:152].unsqueeze(3)
                                .broadcast_to((P, C, H, Dh)),
                            op=OP.mult)
                        # halving trees: even block (+asrc junk+alpha/den),
                        # odd block
                        c = C
                        while c > 1:
                            hh = c // 2
                            nc.vector.tensor_tensor(
                                out=g[:, 0:hh, 0:152], in0=g[:, 0:hh, 0:152],
                                in1=g[:, c - hh:c, 0:152], op=OP.add)
                            nc.vector.tensor_tensor(
                                out=g[:, 0:hh, FXE:FXE + 128],
                                in0=g[:, 0:hh, FXE:FXE + 128],
                                in1=g[:, c - hh:c, FXE:FXE + 128], op=OP.add)
                            c -= hh
                        nc.vector.tensor_add(msg_acc[:], msg_acc[:],
                                             g[:, 0, 0:128])
                        nc.vector.tensor_add(msg_acc[:], msg_acc[:],
                                             g[:, 0, FXE:FXE + 128])
                        nc.vector.tensor_add(den_acc[:], den_acc[:],
                                             g[:, 0, 136:144])
                        nc.vector.tensor_add(den_acc[:], den_acc[:],
                                             g[:, 0, 144:152])
                    # normalize + stats
                    rec = accp.tile([P, H], f32, name="rec", tag="rec")
                    nc.vector.tensor_scalar_add(rec[:], den_acc[:], 1e-16)
                    nc.vector.reciprocal(rec[:], rec[:])
                    op_t = out_all[:, t, :]
                    nc.vector.tensor_tensor(
                        out=op_t.rearrange("p (h d) -> p h d", h=H),
                        in0=msg_acc.rearrange("p (h d) -> p h d", h=H),
                        in1=rec.unsqueeze(2).broadcast_to((P, H, Dh)),
                        op=OP.mult)
                    sq = accp.tile([P, F], f32, name="sq", tag="sq")
                    nc.vector.tensor_mul(sq[:], op_t, op_t)
                    stp = st_ps.tile([P, 2], f32, name="stp", tag="stp")
                    nc.tensor.matmul(out=stp[:, 0:1], lhsT=op_t, rhs=ones[:],
                                     start=True, stop=True)
                    nc.tensor.matmul(out=stp[:, 1:2], lhsT=sq[:], rhs=ones[:],
                                     start=True, stop=True)
                    nc.vector.tensor_add(acc[:], acc[:], stp[:])

            # ---------------- Phase C: stats allreduce + normalize + ELU
            st_in = dram.tile([P, 2], f32)
            st_out = dram.tile([P, 2], f32, addr_space="Shared")
            nc.sync.dma_start(out=st_in[:], in_=acc[:])
            nc.gpsimd.collective_compute(
                "AllReduce", mybir.AluOpType.add,
                replica_groups=[list(range(NC))],
                ins=[st_in[:].opt()], outs=[st_out[:].opt()])
            sg = keep.tile([P, 2], f32)
            nc.sync.dma_start(out=sg[:], in_=st_out[:])
            mean = keep.tile([P, 1], f32)
            nc.vector.tensor_scalar_mul(mean[:], sg[:, 0:1], 1.0 / N)
            ex2 = keep.tile([P, 1], f32)
            nc.vector.tensor_scalar_mul(ex2[:], sg[:, 1:2], 1.0 / N)
            var = keep.tile([P, 1], f32)
            nc.vector.tensor_mul(var[:], mean[:], mean[:])
            nc.vector.tensor_sub(var[:], ex2[:], var[:])
            rstd = keep.tile([P, 1], f32)
            eps_t = keep.tile([P, 1], f32)
            nc.vector.memset(eps_t[:], EPS_IN)
            nc.scalar.activation(out=rstd[:], in_=var[:], func=AF.Sqrt,
                                 bias=eps_t[:])
            nc.vector.reciprocal(rstd[:], rstd[:])
            gam_s = keep.tile([P, 1], f32)
            nc.sync.dma_start(out=gam_s[:], in_=gam_d[:, None])
            bet_s = keep.tile([P, 1], f32)
            nc.sync.dma_start(out=bet_s[:], in_=bet_d[:, None])
            scl = keep.tile([P, 1], f32)
            nc.vector.tensor_mul(scl[:], rstd[:], gam_s[:])
            bia = keep.tile([P, 1], f32)
            nc.vector.tensor_mul(bia[:], mean[:], scl[:])
            nc.vector.tensor_sub(bia[:], bet_s[:], bia[:])
            sb_dram = dram.tile([2, P], f32)
            nc.sync.dma_start(out=sb_dram[0, :], in_=scl[:, 0])
            nc.sync.dma_start(out=sb_dram[1, :], in_=bia[:, 0])
            sclB = keep.tile([P, F], f32)
            nc.sync.dma_start(out=sclB[:],
                              in_=sb_dram[0:1, :].broadcast_to((P, P)))
            biaB = keep.tile([P, F], f32)
            nc.sync.dma_start(out=biaB[:],
                              in_=sb_dram[1:2, :].broadcast_to((P, P)))

            with tc.tile_pool(name="phc", bufs=2) as phc:
                TB = 13
                for t0 in range(0, n_tiles, TB):
                    nt = min(TB, n_tiles - t0)
                    xo_t = phc.tile([P, TB, F], f32, name="xo_t")
                    nc.sync.dma_start(
                        out=xo_t[:, 0:nt, :],
                        in_=xo_d[t0 * P:(t0 + nt) * P, :]
                            .rearrange("(c p) f -> p c f", p=P))
                    z = phc.tile([P, TB, F], f32, name="z")
                    nc.vector.tensor_tensor(
                        out=z[:, 0:nt, :], in0=out_all[:, t0:t0 + nt, :],
                        in1=sclB.unsqueeze(1).broadcast_to((P, nt, F)),
                        op=OP.mult)
                    nc.vector.tensor_tensor(
                        out=z[:, 0:nt, :], in0=z[:, 0:nt, :],
                        in1=biaB.unsqueeze(1).broadcast_to((P, nt, F)),
                        op=OP.add)
                    nc.vector.tensor_add(z[:, 0:nt, :], z[:, 0:nt, :],
                                         xo_t[:, 0:nt, :])
                    zf = z[:, 0:nt, :].rearrange("p c f -> p (c f)")
                    zm = phc.tile([P, TB, F], f32, name="zm")
                    zmf = zm[:, 0:nt, :].rearrange("p c f -> p (c f)")
                    nc.vector.tensor_scalar_min(zmf, zf, 0.0)
                    nc.scalar.activation(out=zmf, in_=zmf, func=AF.Exp)
                    nc.vector.tensor_scalar_max(zf, zf, 0.0)
                    nc.vector.tensor_add(zf, zf, zmf)
                    nc.vector.tensor_scalar_add(zf, zf, -1.0)
                    nc.scalar.dma_start(
                        out=out_d[t0 * P:(t0 + nt) * P, :]
                            .rearrange("(c p) f -> p c f", p=P),
                        in_=z[:, 0:nt, :])
    if finalize:
        nc.finalize()
    return nc


# ---------------------------------------------------------------- driver
def _run_gat(x, edge_index, edge_attr, W, att_src, att_dst, W_e, att_edge,
             gamma, beta, cfg, trace=False, return_results=False, sim=False):
    N, NC = cfg["N"], cfg["NC"]
    Np = N // NC
    Wb, v8 = _fold_weights(
        np.asarray(W, np.float32), np.asarray(att_src, np.float32),
        np.asarray(att_dst, np.float32), np.asarray(W_e, np.float32),
        np.asarray(att_edge, np.float32))
    cores, nodes_of_core, xT_pa, meta = _preprocess(x, edge_index, edge_attr,
                                                    cfg)
    nc = _build(meta)

    gam = np.asarray(gamma, np.float32)
    bet = np.asarray(beta, np.float32)
    n_tiles = meta["n_tiles"]
    in_maps = []
    for c in range(NC):
        sti = cores[c]["in"]
        in_maps.append(dict(
            xT=xT_pa, xTo=sti["xTo"], xo=sti["xo"], Wb=Wb, v8=v8,
            idx=sti["idx"], mask=sti["mask"], ea8=sti["ea8"],
            rdeg=sti["rdeg"], gamma=gam, beta=bet))
    if sim:
        from concourse.bass_interp import MultiCoreSim
        ms = MultiCoreSim(nc, num_cores=NC)
        for c, cs in ms.cores.items():
            for k, v in in_maps[c].items():
                cs.tensor(k)[:] = v
        ms.simulate()
        results = [{"out": np.asarray(ms.cores[c].tensor("out"))}
                   for c in range(NC)]
        res = None
    else:
        from concourse.bass_utils import run_bass_kernel_spmd
        res = run_bass_kernel_spmd(nc, in_maps, core_ids=list(range(NC)),
                                   trace=trace)
        results = res.results
    out = np.empty((N, F), dtype=np.float32)
    for c in range(NC):
        oc = results[c]["out"]
        out[nodes_of_core[c]] = oc[:Np]
    if return_results:
        return out, res
    return out


def kernel(x, edge_index, edge_attr, W, att_src, att_dst, W_e, att_edge,
           gamma, beta):
    return _run_gat(x, edge_index, edge_attr, W, att_src, att_dst, W_e,
                    att_edge, gamma, beta, _cfg_full())


# revision 3
# speedup vs baseline: 1.0188x; 1.0170x over previous
"""GAT block (GATConv + InstanceNorm + residual + ELU) on 8 Trainium2 cores.

v2 strategy (gather-ucode-minimal):
  - dst-node graph parallel across 8 cores; nodes snake-dealt to cores by
    global in-degree, then deg-sorted into 128-node tiles so per-tile max
    degree (= slot columns) is minimal and aligned across cores.
  - ONE gather index per edge: the DRAM table packs NODE PAIRS per row
    (768B: [h(2k) bf16 128 | asrc(2k) | pad | h(2k+1) | asrc(2k+1) | pad]),
    so idx = src>>1 fits int16 with no A/B table split. A host-built
    {0,-1e30} mask picks the even/odd half in the logits (wrong half's
    alpha underflows to 0).
  - per-tile slot layout [dst=128 partitions, slot cols, 384 bf16]; alpha
    written into the row's pad region so one halving-tree accumulates
    messages AND softmax denominators; all DVE ops bf16/contiguous.
  - a_edge via TensorE on host-packed 8-slot-interleaved eaT8 (bf16);
    self-loop edge_attr = mean of incoming, via linearity.
  - InstanceNorm stats via ones-matmul + AllReduce; finalize = affine +
    residual + ELU (fp32).
"""

import math
import numpy as np

P = 128
F, H, Dh, ED = 128, 8, 16, 16
FXE = 192          # bf16 elems per node block in a table row
ROWW = 2 * FXE     # pair row width (384 bf16 = 768B)
KCAP = 40          # max edge slot-cols per chunk
GMAX = 8           # slot-cols per gather instruction (1024 idxs)
EPS_IN, NEG, MNEG = 1e-5, 0.2, -1e30


def _cfg_full():
    return dict(N=50000, E=1600000, NC=8)


def _fold_weights(W, att_src, att_dst, W_e, att_edge):
    import ml_dtypes
    w_src = np.stack(
        [W[:, h * Dh:(h + 1) * Dh] @ att_src[h] for h in range(H)], axis=1)
    w_dst = np.stack(
        [W[:, h * Dh:(h + 1) * Dh] @ att_dst[h] for h in range(H)], axis=1)
    Wb = np.concatenate([W, w_src, w_dst], axis=1)  # [F, 144]
    v = np.stack(
        [W_e[:, h * Dh:(h + 1) * Dh] @ att_edge[h] for h in range(H)], axis=1)
    v8 = np.zeros((8 * ED, 8 * H), dtype=np.float32)
    for s in range(8):
        v8[s * ED:(s + 1) * ED, s * H:(s + 1) * H] = v
    return Wb.astype(ml_dtypes.bfloat16), v8.astype(ml_dtypes.bfloat16)


def _chunks_of(K):
    """Chunk list for a tile with K edge slots: [(j0, ne, has_self), ...].
    Chunk 0 (with the self col) is listed first; device processes it LAST."""
    ch = [(0, min(K, KCAP - 1), True)]
    j = KCAP - 1
    while j < K:
        ch.append((j, min(KCAP, K - j), False))
        j += KCAP
    return ch


def _pack16(flat):
    cols = len(flat) // 16
    out2 = np.zeros((P, max(cols, 1)), dtype=np.int16)
    if cols:
        out2[:] = np.tile(flat.reshape(-1, 16).T, (8, 1))
    return out2


def _preprocess(x, edge_index, edge_attr, cfg):
    import ml_dtypes
    N, E, NC = cfg["N"], cfg["E"], cfg["NC"]
    Np = N // NC
    n_tiles = math.ceil(Np / P)
    src = np.asarray(edge_index[0]).astype(np.int64)
    dst = np.asarray(edge_index[1]).astype(np.int64)
    ea = np.asarray(edge_attr, dtype=np.float32)
    x_np = np.asarray(x, dtype=np.float32)

    # ---- node -> (core, tile, partition): global-degree snake deal
    deg_g = np.bincount(dst, minlength=N)
    order = np.argsort(-deg_g, kind="stable")
    ranks = np.arange(N)
    blk, pos = ranks // NC, ranks % NC
    core_of_rank = np.where(blk % 2 == 0, pos, NC - 1 - pos)
    assign = np.empty(N, dtype=np.int64)
    assign[order] = core_of_rank
    local_rank = np.empty(N, dtype=np.int64)
    nodes_of_core = []
    for c in range(NC):
        nodes_c = order[core_of_rank == c]          # deg-desc order
        assert len(nodes_c) == Np
        local_rank[nodes_c] = np.arange(Np)
        nodes_of_core.append(nodes_c)

    # ---- per-core edge routing and per-tile max degree
    cores = []
    Kct = np.zeros((NC, n_tiles), dtype=np.int64)
    for c in range(NC):
        m = assign[dst] == c
        e_ids = np.nonzero(m)[0]
        dl = local_rank[dst[e_ids]]
        o = np.argsort(dl, kind="stable")
        e_ids, dl = e_ids[o], dl[o]
        deg = np.bincount(dl, minlength=Np)
        cum = np.zeros(Np + 1, dtype=np.int64)
        np.cumsum(deg, out=cum[1:])
        j_e = np.arange(len(dl)) - cum[dl]
        t_e, p_e = dl // P, dl % P
        np.maximum.at(Kct[c], t_e, j_e + 1)
        cores.append(dict(e_ids=e_ids, dl=dl, j=j_e, t=t_e, p=p_e, deg=deg))

    K_t = Kct.max(axis=0)

    # ---- shared chunk schedule + offsets (identical across cores)
    chunks = []          # (t, j0, ne, has_self, C, EQ)
    for t in range(n_tiles):
        for (j0, ne, hs) in _chunks_of(int(K_t[t])):
            C = ne + (1 if hs else 0)
            EQ = (ne + 7) // 8
            chunks.append((t, j0, ne, hs, C, EQ))
    n_chunks = len(chunks)
    idx_off = np.zeros(n_chunks + 1, dtype=np.int64)   # in idxs
    mask_off = np.zeros(n_chunks + 1, dtype=np.int64)  # in cols (per partition)
    ea_off = np.zeros(n_chunks + 1, dtype=np.int64)    # in cols
    for i, (t, j0, ne, hs, C, EQ) in enumerate(chunks):
        idx_off[i + 1] = idx_off[i] + ne * P
        mask_off[i + 1] = mask_off[i] + C * 16
        ea_off[i + 1] = ea_off[i] + EQ * P
    chunk_no_of = {}
    for i, (t, j0, ne, hs, C, EQ) in enumerate(chunks):
        chunk_no_of[(t, j0)] = i

    # chunk id lookup for an edge slot j: piecewise
    def _ci_arrays(j):
        in0 = j < (KCAP - 1)
        ci = np.where(in0, 0, 1 + (j - (KCAP - 1)) // KCAP)
        j0 = np.where(in0, 0, (KCAP - 1) + ((j - (KCAP - 1)) // KCAP) * KCAP)
        jj = j - j0
        cc = jj + np.where(in0, 1, 0)   # col within chunk (self col shifts)
        return ci, j0, jj, cc

    ea_bf = ea.astype(ml_dtypes.bfloat16)
    SIDX = int(idx_off[-1])
    SMASK = int(mask_off[-1])
    SEA = int(ea_off[-1])

    # tile-major base chunk numbers
    for c in range(NC):
        st = cores[c]
        t_e, p_e, j_e = st["t"], st["p"], st["j"]
        src_e = src[st["e_ids"]]
        ci, j0, jj, cc = _ci_arrays(j_e)
        # vectorized chunk_no: build lookup [n_tiles, max_ci]
        max_ci = 1 + max(0, (int(K_t.max()) - (KCAP - 1) + KCAP - 1) // KCAP)
        lut = np.full((n_tiles, max_ci + 1), -1, dtype=np.int64)
        for i, (t, jj0, ne, hs, C, EQ) in enumerate(chunks):
            cidx = 0 if hs else 1 + (jj0 - (KCAP - 1)) // KCAP
            lut[t, cidx] = i
        cno = lut[t_e, ci]
        assert (cno >= 0).all()

        idxA = np.zeros(SIDX, dtype=np.int16)
        idxA[idx_off[cno] + jj * P + p_e] = (src_e >> 1).astype(np.int16)
        maskA = np.full((P, SMASK), MNEG, dtype=ml_dtypes.bfloat16)
        colm = (mask_off[cno] + cc * 16 + (src_e & 1) * 8).astype(np.int64)
        maskA[p_e[:, None], colm[:, None] + np.arange(8)[None, :]] = 0.0
        # self cols: even half active
        for i, (t, jj0, ne, hs, C, EQ) in enumerate(chunks):
            if hs:
                maskA[:, int(mask_off[i]):int(mask_off[i]) + 8] = 0.0
        ea8 = np.zeros((8 * ED, SEA), dtype=ml_dtypes.bfloat16)
        q_e, s_e = jj // 8, jj % 8
        cole = (ea_off[cno] + q_e * P + p_e).astype(np.int64)
        rows = (s_e[:, None] * ED + np.arange(ED)[None, :]).astype(np.int64)
        ea8[rows, cole[:, None]] = ea_bf[st["e_ids"]]

        rdeg = np.ones((P, n_tiles), dtype=np.float32)
        deg = st["deg"]
        idxs = np.arange(Np)
        rdeg[idxs % P, idxs // P] = 1.0 / np.maximum(deg, 1.0)

        nodes_c = nodes_of_core[c]
        pad = n_tiles * P - Np
        xo = np.zeros((n_tiles * P, F), dtype=np.float32)
        xo[:Np] = x_np[nodes_c]
        xTo = np.ascontiguousarray(xo.T).astype(ml_dtypes.bfloat16)
        st["in"] = dict(idx=_pack16(idxA), mask=maskA, ea8=ea8, rdeg=rdeg,
                        xo=xo, xTo=xTo)

    # pair-interleaved xT for Phase A (shared by all cores); evens at
    # partitions 0..63, odds at 64..127 of each 128-node chunk, zero-padded
    n_chunksA = math.ceil(N / P)
    xpad = np.zeros((n_chunksA * P, F), dtype=np.float32)
    for i0 in range(0, N, P):
        nrow = min(P, N - i0)
        assert nrow % 2 == 0
        xpad[i0:i0 + nrow // 2] = x_np[i0:i0 + nrow:2]
        xpad[i0 + 64:i0 + 64 + nrow // 2] = x_np[i0 + 1:i0 + nrow:2]
    xT_pa = np.ascontiguousarray(xpad.T).astype(ml_dtypes.bfloat16)

    meta = dict(N=N, NC=NC, Np=Np, n_tiles=n_tiles, K_t=K_t, chunks=chunks,
                idx_off=idx_off, mask_off=mask_off, ea_off=ea_off,
                SIDX=SIDX, SMASK=SMASK, SEA=SEA)
    return cores, nodes_of_core, xT_pa, meta


# ---------------------------------------------------------------- device
def _build(meta, finalize=True):
    import concourse.bass as bass
    import concourse.bacc as bacc
    import concourse.tile as tile
    from concourse import mybir

    N, NC = meta["N"], meta["NC"]
    n_tiles = meta["n_tiles"]
    chunks = meta["chunks"]
    idx_off, mask_off, ea_off = meta["idx_off"], meta["mask_off"], meta["ea_off"]
    SIDX, SMASK, SEA = meta["SIDX"], meta["SMASK"], meta["SEA"]
    NPAIR = N // 2
    f32 = mybir.dt.float32
    bf16 = mybir.dt.bfloat16
    i16 = mybir.dt.int16
    AF = mybir.ActivationFunctionType
    OP = mybir.AluOpType
    KMAX = int(max(c[4] for c in chunks))   # max C

    n_chunksA = math.ceil(N / P)
    nc = bacc.Bacc("TRN2", target_bir_lowering=False, debug=False,
                   num_devices=NC)
    xT_d = nc.declare_dram_parameter("xT", [F, n_chunksA * P], bf16,
                                     isOutput=False)
    xTo_d = nc.declare_dram_parameter("xTo", [F, n_tiles * P], bf16,
                                      isOutput=False)
    xo_d = nc.declare_dram_parameter("xo", [n_tiles * P, F], f32,
                                     isOutput=False)
    Wb_d = nc.declare_dram_parameter("Wb", [F, 144], bf16, isOutput=False)
    v8_d = nc.declare_dram_parameter("v8", [8 * ED, 8 * H], bf16,
                                     isOutput=False)
    ix_d = nc.declare_dram_parameter("idx", [P, max(SIDX // 16, 1)], i16,
                                     isOutput=False)
    mk_d = nc.declare_dram_parameter("mask", [P, SMASK], bf16, isOutput=False)
    ea_d = nc.declare_dram_parameter("ea8", [8 * ED, SEA], bf16,
                                     isOutput=False)
    rdeg_d = nc.declare_dram_parameter("rdeg", [P, n_tiles], f32,
                                       isOutput=False)
    gam_d = nc.declare_dram_parameter("gamma", [F], f32, isOutput=False)
    bet_d = nc.declare_dram_parameter("beta", [F], f32, isOutput=False)
    out_d = nc.declare_dram_parameter("out", [n_tiles * P, F], f32,
                                      isOutput=True)

    with tile.TileContext(nc) as tc:
        with (
            tc.tile_pool(name="dram", bufs=1, space="DRAM") as dram,
            tc.tile_pool(name="consts", bufs=1) as consts,
            tc.tile_pool(name="keep", bufs=1) as keep,
        ):
            hx = dram.tile([NPAIR, ROWW], bf16)

            Wb_s = consts.tile([F, 144], bf16)
            nc.sync.dma_start(out=Wb_s[:], in_=Wb_d[:, :])
            v8_s = consts.tile([8 * ED, 8 * H], bf16)
            nc.sync.dma_start(out=v8_s[:], in_=v8_d[:, :])
            ones = consts.tile([P, 1], f32)
            nc.vector.memset(ones[:], 1.0)
            rdeg_s = consts.tile([P, n_tiles], f32)
            nc.sync.dma_start(out=rdeg_s[:], in_=rdeg_d[:, :])

            hx_own = keep.tile([P, n_tiles, 144], bf16)
            out_all = keep.tile([P, n_tiles, F], f32)
            acc = keep.tile([P, 2], f32)
            nc.vector.memset(acc[:], 0.0)

            # ---------------- Phase A: pair table hx = x @ Wb
            # 8 node-chunks share one staging tile -> 2 bulk DMAs per group
            with (
                tc.tile_pool(name="pha", bufs=4) as pha,
                tc.tile_pool(name="pha_ps", bufs=4, space="PSUM") as pha_ps,
            ):
                CB = 16
                n_full = N // P           # full 128-node chunks
                for i0 in range(0, n_chunksA, CB):
                    nb = min(CB, n_chunksA - i0)
                    bulk = (i0 + nb <= n_full)   # all chunks full-size
                    st8 = pha.tile([P, CB, FXE], bf16, name="st8", tag="st8")
                    nc.vector.memset(st8[:, :, 136:FXE], 0.0)
                    for j in range(0, nb, 8):
                        nx = min(8, nb - j)
                        xT_t = pha.tile([F, 8 * P], bf16, name="xT_t",
                                        tag="xT_t")
                        nc.scalar.dma_start(
                            out=xT_t[:, 0:nx * P],
                            in_=xT_d[:, (i0 + j) * P:(i0 + j + nx) * P])
                        for k in range(nx):
                            hp = pha_ps.tile([P, 144], f32, name="hp",
                                             tag="hp")
                            nc.tensor.matmul(out=hp[:],
                                             lhsT=xT_t[:, k * P:(k + 1) * P],
                                             rhs=Wb_s[:],
                                             start=True, stop=True)
                            nc.vector.tensor_copy(
                                out=st8[:, j + k, 0:136], in_=hp[:, 0:136])
                    prg = i0 * 64
                    if bulk:
                        nc.sync.dma_start(
                            out=hx[prg:prg + nb * 64, 0:FXE]
                                .rearrange("(c p) f -> p c f", p=64),
                            in_=st8[0:64, 0:nb, :])
                        nc.scalar.dma_start(
                            out=hx[prg:prg + nb * 64, FXE:ROWW]
                                .rearrange("(c p) f -> p c f", p=64),
                            in_=st8[64:128, 0:nb, :])
                    else:
                        for j in range(nb):
                            r0 = (i0 + j) * P
                            npair = min(P, N - r0) // 2
                            pr0 = r0 // 2
                            nc.sync.dma_start(
                                out=hx[pr0:pr0 + npair, 0:FXE],
                                in_=st8[0:npair, j, :])
                            nc.scalar.dma_start(
                                out=hx[pr0:pr0 + npair, FXE:ROWW],
                                in_=st8[64:64 + npair, j, :])
                # own nodes (tile order): h | asrc | adst, fp32
                for t in range(n_tiles):
                    xTo_t = pha.tile([F, P], bf16, name="xTo_t", tag="xT_t")
                    nc.sync.dma_start(out=xTo_t[:],
                                      in_=xTo_d[:, t * P:(t + 1) * P])
                    hp = pha_ps.tile([P, 144], f32, name="hp2", tag="hp")
                    nc.tensor.matmul(out=hp[:], lhsT=xTo_t[:], rhs=Wb_s[:],
                                     start=True, stop=True)
                    nc.vector.tensor_copy(out=hx_own[:, t, :], in_=hp[:])

            # ---------------- Phase B: per-tile attention + aggregation
            with (
                tc.tile_pool(name="phb", bufs=3) as phb,
                tc.tile_pool(name="acc_p", bufs=2) as accp,
                tc.tile_pool(name="phb_ps", bufs=4, space="PSUM") as phb_ps,
                tc.tile_pool(name="st_ps", bufs=2, space="PSUM") as st_ps,
            ):
                for t in range(n_tiles):
                    tile_chunks = [ch for ch in chunks if ch[0] == t]
                    # process non-self chunks first, self chunk last
                    tile_chunks = ([c for c in tile_chunks if not c[3]]
                                   + [c for c in tile_chunks if c[3]])
                    msg_acc = accp.tile([P, F], f32, name="msg_acc",
                                        tag="msg_acc")
                    den_acc = accp.tile([P, H], f32, name="den_acc",
                                        tag="den_acc")
                    aeL_acc = accp.tile([P, H], f32, name="aeL_acc",
                                        tag="aeL_acc")
                    nc.vector.memset(msg_acc[:], 0.0)
                    nc.vector.memset(den_acc[:], 0.0)
                    nc.vector.memset(aeL_acc[:], 0.0)
                    for (tt, j0, ne, hs, C, EQ) in tile_chunks:
                        cno = None
                        for i, ch in enumerate(chunks):
                            if ch[0] == t and ch[1] == j0:
                                cno = i
                                break
                        e0 = 1 if hs else 0
                        g = phb.tile([P, KMAX, ROWW], bf16, name="g", tag="g")
                        mk = phb.tile([P, KMAX, 16], bf16, name="mk", tag="mk")
                        nc.scalar.dma_start(
                            out=mk[:, 0:C, :].rearrange("p c h -> p (c h)"),
                            in_=mk_d[:, int(mask_off[cno]):int(mask_off[cno + 1])])
                        if ne:
                            ixt = phb.tile([P, KCAP * 8], i16, name="ixt",
                                           tag="ixt")
                            o0 = int(idx_off[cno]) // 16
                            nc.scalar.dma_start(out=ixt[:, 0:ne * 8],
                                                in_=ix_d[:, o0:o0 + ne * 8])
                            ea8 = phb.tile([8 * ED, (KCAP // 8) * P], bf16,
                                           name="ea8", tag="ea8")
                            nc.sync.dma_start(
                                out=ea8[:, 0:EQ * P],
                                in_=ea_d[:, int(ea_off[cno]):int(ea_off[cno + 1])])
                        if hs:
                            nc.vector.tensor_copy(out=g[:, 0, 0:136],
                                                  in_=hx_own[:, t, 0:136])
                            nc.vector.tensor_copy(out=g[:, 0, FXE:FXE + 136],
                                                  in_=hx_own[:, t, 0:136])
                        for g0 in range(0, ne, GMAX):
                            kk = min(GMAX, ne - g0)
                            nc.gpsimd.dma_gather(
                                out_ap=g[:, e0 + g0:e0 + g0 + kk, :],
                                in_ap=hx[:, :],
                                idxs_ap=ixt[:, g0 * 8:(g0 + kk) * 8],
                                num_idxs=kk * P,
                                num_idxs_reg=kk * P,
                                elem_size=ROWW,
                            )
                        # a_edge
                        ae = phb.tile([P, KMAX, H], f32, name="ae", tag="ae")
                        for q in range(EQ):
                            aep = phb_ps.tile([P, 8 * H], f32, name="aep",
                                              tag="aep")
                            nc.tensor.matmul(
                                out=aep[:],
                                lhsT=ea8[:, q * P:(q + 1) * P],
                                rhs=v8_s[:], start=True, stop=True)
                            nq = min(8, ne - q * 8)
                            nc.vector.tensor_copy(
                                out=ae[:, e0 + q * 8:e0 + q * 8 + nq, :]
                                    .rearrange("p c h -> p (c h)"),
                                in_=aep[:, 0:nq * H])
                        if ne:
                            aeL = phb.tile([P, H], f32, name="aeL", tag="aeL")
                            nc.vector.tensor_reduce(
                                out=aeL[:],
                                in_=ae[:, e0:e0 + ne, :].transpose([0, 2, 1]),
                                axis=mybir.AxisListType.X, op=OP.add)
                            nc.vector.tensor_add(aeL_acc[:], aeL_acc[:],
                                                 aeL[:])
                        if hs:
                            nc.vector.tensor_scalar_mul(
                                ae[:, 0, :], aeL_acc[:], rdeg_s[:, t:t + 1])
                        # logits [P, C, 16] fp32
                        al = phb.tile([P, KMAX, 16], f32, name="al", tag="al")
                        adst_b = hx_own[:, t, 136:144].unsqueeze(1) \
                            .broadcast_to((P, C, H))
                        nc.vector.tensor_tensor(
                            out=al[:, 0:C, 0:8], in0=ae[:, 0:C, :],
                            in1=adst_b, op=OP.add)
                        nc.vector.tensor_tensor(
                            out=al[:, 0:C, 8:16], in0=ae[:, 0:C, :],
                            in1=adst_b, op=OP.add)
                        nc.vector.tensor_tensor(
                            out=al[:, 0:C, 0:8], in0=al[:, 0:C, 0:8],
                            in1=g[:, 0:C, 128:136], op=OP.add)
                        nc.vector.tensor_tensor(
                            out=al[:, 0:C, 8:16], in0=al[:, 0:C, 8:16],
                            in1=g[:, 0:C, FXE + 128:FXE + 136], op=OP.add)
                        nc.vector.tensor_tensor(
                            out=al[:, 0:C, :], in0=al[:, 0:C, :],
                            in1=mk[:, 0:C, :], op=OP.add)
                        nc.vector.scalar_tensor_tensor(
                            out=al[:, 0:C, :], in0=al[:, 0:C, :], scalar=NEG,
                            in1=al[:, 0:C, :], op0=OP.mult, op1=OP.max)
                        nc.vector.tensor_scalar_max(al[:, 0:C, :],
                                                    al[:, 0:C, :], -88.0)
                        nc.scalar.activation(out=g[:, 0:C, 136:152],
                                             in_=al[:, 0:C, :], func=AF.Exp)
                        # fold alpha into h (both halves)
                        nc.vector.tensor_tensor(
                            out=g[:, 0:C, 0:128].rearrange(
                                "p c (h d) -> p c h d", h=H),
                            in0=g[:, 0:C, 0:128].rearrange(
                                "p c (h d) -> p c h d", h=H),
                            in1=g[:, 0:C, 136:144].unsqueeze(3)
                                .broadcast_to((P, C, H, Dh)),
                            op=OP.mult)
                        nc.vector.tensor_tensor(
                            out=g[:, 0:C, FXE:FXE + 128].rearrange(
                                "p c (h d) -> p c h d", h=H),
                            in0=g[:, 0:C, FXE:FXE + 128].rearrange(
                                "p c (h d) -> p c h d", h=H),
                            in1=g[:, 0:C, 144:152].unsqueeze(3)
                                .broadcast_to((P, C, H, Dh)),
                            op=OP.mult)
                        # halving trees: even block (+asrc junk+alpha/den),
                        # odd block
                        c = C
                        while c > 1:
                            hh = c // 2
                            nc.vector.tensor_tensor(
                                out=g[:, 0:hh, 0:152], in0=g[:, 0:hh, 0:152],
                                in1=g[:, c - hh:c, 0:152], op=OP.add)
                            nc.vector.tensor_tensor(
                                out=g[:, 0:hh, FXE:FXE + 128],
                                in0=g[:, 0:hh, FXE:FXE + 128],
                                in1=g[:, c - hh:c, FXE:FXE + 128], op=OP.add)
                            c -= hh
                        nc.vector.tensor_add(msg_acc[:], msg_acc[:],
                                             g[:, 0, 0:128])
                        nc.vector.tensor_add(msg_acc[:], msg_acc[:],
                                             g[:, 0, FXE:FXE + 128])
                        nc.vector.tensor_add(den_acc[:], den_acc[:],
                                             g[:, 0, 136:144])
                        nc.vector.tensor_add(den_acc[:], den_acc[:],
                                             g[:, 0, 144:152])
                    # normalize + stats
                    rec = accp.tile([P, H], f32, name="rec", tag="rec")
                    nc.vector.tensor_scalar_add(rec[:], den_acc[:], 1e-16)
                    nc.vector.reciprocal(rec[:], rec[:])
                    op_t = out_all[:, t, :]
                    nc.vector.tensor_tensor(
                        out=op_t.rearrange("p (h d) -> p h d", h=H),
                        in0=msg_acc.rearrange("p (h d) -> p h d", h=H),
                        in1=rec.unsqueeze(2).broadcast_to((P, H, Dh)),
                        op=OP.mult)
                    sq = accp.tile([P, F], f32, name="sq", tag="sq")
                    nc.vector.tensor_mul(sq[:], op_t, op_t)
                    stp = st_ps.tile([P, 2], f32, name="stp", tag="stp")
                    nc.tensor.matmul(out=stp[:, 0:1], lhsT=op_t, rhs=ones[:],
                                     start=True, stop=True)
                    nc.tensor.matmul(out=stp[:, 1:2], lhsT=sq[:], rhs=ones[:],
                                     start=True, stop=True)
                    nc.vector.tensor_add(acc[:], acc[:], stp[:])

            # ---------------- Phase C: stats allreduce + normalize + ELU
            st_in = dram.tile([P, 2], f32)
            st_out = dram.tile([P, 2], f32, addr_space="Shared")
            nc.sync.dma_start(out=st_in[:], in_=acc[:])
            nc.gpsimd.collective_compute(
                "AllReduce", mybir.AluOpType.add,
                replica_groups=[list(range(NC))],
                ins=[st_in[:].opt()], outs=[st_out[:].opt()])
            sg = keep.tile([P, 2], f32)
            nc.sync.dma_start(out=sg[:], in_=st_out[:])
            mean = keep.tile([P, 1], f32)
            nc.vector.tensor_scalar_mul(mean[:], sg[:, 0:1], 1.0 / N)
            ex2 = keep.tile([P, 1], f32)
            nc.vector.tensor_scalar_mul(ex2[:], sg[:, 1:2], 1.0 / N)
            var = keep.tile([P, 1], f32)
            nc.vector.tensor_mul(var[:], mean[:], mean[:])
            nc.vector.tensor_sub(var[:], ex2[:], var[:])
            rstd = keep.tile([P, 1], f32)
            eps_t = keep.tile([P, 1], f32)
            nc.vector.memset(eps_t[:], EPS_IN)
            nc.scalar.activation(out=rstd[:], in_=var[:], func=AF.Sqrt,
                                 bias=eps_t[:])
            nc.vector.reciprocal(rstd[:], rstd[:])
            gam_s = keep.tile([P, 1], f32)
            nc.sync.dma_start(out=gam_s[:], in_=gam_d[:, None])
            bet_s = keep.tile([P, 1], f32)
            nc.sync.dma_start(out=bet_s[:], in_=bet_d[:, None])
            scl = keep.tile([P, 1], f32)
            nc.vector.tensor_mul(scl[:], rstd[:], gam_s[:])
            bia = keep.tile([P, 1], f32)
            nc.vector.tensor_mul(bia[:], mean[:], scl[:])
            nc.vector.tensor_sub(bia[:], bet_s[:], bia[:])
            sb_dram = dram.tile([2, P], f32)
            nc.sync.dma_start(out=sb_dram[0, :], in_=scl[:, 0])
            nc.sync.dma_start(out=sb_dram[1, :], in_=bia[:, 0])
            sclB = keep.tile([P, F], f32)
            nc.sync.dma_start(out=sclB[:],
                              in_=sb_dram[0:1, :].broadcast_to((P, P)))
            biaB = keep.tile([P, F], f32)
            nc.sync.dma_start(out=biaB[:],
                              in_=sb_dram[1:2, :].broadcast_to((P, P)))

            with tc.tile_pool(name="phc", bufs=2) as phc:
                TB = 13
                for t0 in range(0, n_tiles, TB):
                    nt = min(TB, n_tiles - t0)
                    xo_t = phc.tile([P, TB, F], f32, name="xo_t")
                    nc.sync.dma_start(
                        out=xo_t[:, 0:nt, :],
                        in_=xo_d[t0 * P:(t0 + nt) * P, :]
                            .rearrange("(c p) f -> p c f", p=P))
                    z = phc.tile([P, TB, F], f32, name="z")
                    nc.vector.tensor_tensor(
                        out=z[:, 0:nt, :], in0=out_all[:, t0:t0 + nt, :],
                        in1=sclB.unsqueeze(1).broadcast_to((P, nt, F)),
                        op=OP.mult)
                    nc.vector.tensor_tensor(
                        out=z[:, 0:nt, :], in0=z[:, 0:nt, :],
                        in1=biaB.unsqueeze(1).broadcast_to((P, nt, F)),
                        op=OP.add)
                    nc.vector.tensor_add(z[:, 0:nt, :], z[:, 0:nt, :],
                                         xo_t[:, 0:nt, :])
                    zf = z[:, 0:nt, :].rearrange("p c f -> p (c f)")
                    zm = phc.tile([P, TB, F], f32, name="zm")
                    zmf = zm[:, 0:nt, :].rearrange("p c f -> p (c f)")
                    nc.vector.tensor_scalar_min(zmf, zf, 0.0)
                    nc.scalar.activation(out=zmf, in_=zmf, func=AF.Exp)
                    nc.vector.tensor_scalar_max(zf, zf, 0.0)
                    nc.vector.tensor_add(zf, zf, zmf)
                    nc.vector.tensor_scalar_add(zf, zf, -1.0)
                    nc.scalar.dma_start(
                        out=out_d[t0 * P:(t0 + nt) * P, :]
                            .rearrange("(c p) f -> p c f", p=P),
                        in_=z[:, 0:nt, :])
    if finalize:
        nc.finalize()
    return nc


# ---------------------------------------------------------------- driver
def _run_gat(x, edge_index, edge_attr, W, att_src, att_dst, W_e, att_edge,
             gamma, beta, cfg, trace=False, return_results=False, sim=False):
    N, NC = cfg["N"], cfg["NC"]
    Np = N // NC
    Wb, v8 = _fold_weights(
        np.asarray(W, np.float32), np.asarray(att_src, np.float32),
        np.asarray(att_dst, np.float32), np.asarray(W_e, np.float32),
        np.asarray(att_edge, np.float32))
    cores, nodes_of_core, xT_pa, meta = _preprocess(x, edge_index, edge_attr,
                                                    cfg)
    nc = _build(meta)

    gam = np.asarray(gamma, np.float32)
    bet = np.asarray(beta, np.float32)
    n_tiles = meta["n_tiles"]
    in_maps = []
    for c in range(NC):
        sti = cores[c]["in"]
        in_maps.append(dict(
            xT=xT_pa, xTo=sti["xTo"], xo=sti["xo"], Wb=Wb, v8=v8,
            idx=sti["idx"], mask=sti["mask"], ea8=sti["ea8"],
            rdeg=sti["rdeg"], gamma=gam, beta=bet))
    if sim:
        from concourse.bass_interp import MultiCoreSim
        ms = MultiCoreSim(nc, num_cores=NC)
        for c, cs in ms.cores.items():
            for k, v in in_maps[c].items():
                cs.tensor(k)[:] = v
        ms.simulate()
        results = [{"out": np.asarray(ms.cores[c].tensor("out"))}
                   for c in range(NC)]
        res = None
    else:
        from concourse.bass_utils import run_bass_kernel_spmd
        res = run_bass_kernel_spmd(nc, in_maps, core_ids=list(range(NC)),
                                   trace=trace)
        results = res.results
    out = np.empty((N, F), dtype=np.float32)
    for c in range(NC):
        oc = results[c]["out"]
        out[nodes_of_core[c]] = oc[:Np]
    if return_results:
        return out, res
    return out


def kernel(x, edge_index, edge_attr, W, att_src, att_dst, W_e, att_edge,
           gamma, beta):
    return _run_gat(x, edge_index, edge_attr, W, att_src, att_dst, W_e,
                    att_edge, gamma, beta, _cfg_full())


# revision 4
# speedup vs baseline: 1.0213x; 1.0025x over previous
"""GAT block (GATConv + InstanceNorm + residual + ELU) on 8 Trainium2 cores.

v2 strategy (gather-ucode-minimal):
  - dst-node graph parallel across 8 cores; nodes snake-dealt to cores by
    global in-degree, then deg-sorted into 128-node tiles so per-tile max
    degree (= slot columns) is minimal and aligned across cores.
  - ONE gather index per edge: the DRAM table packs NODE PAIRS per row
    (768B: [h(2k) bf16 128 | asrc(2k) | pad | h(2k+1) | asrc(2k+1) | pad]),
    so idx = src>>1 fits int16 with no A/B table split. A host-built
    {0,-1e30} mask picks the even/odd half in the logits (wrong half's
    alpha underflows to 0).
  - per-tile slot layout [dst=128 partitions, slot cols, 384 bf16]; alpha
    written into the row's pad region so one halving-tree accumulates
    messages AND softmax denominators; all DVE ops bf16/contiguous.
  - a_edge via TensorE on host-packed 8-slot-interleaved eaT8 (bf16);
    self-loop edge_attr = mean of incoming, via linearity.
  - InstanceNorm stats via ones-matmul + AllReduce; finalize = affine +
    residual + ELU (fp32).
"""

import math
import numpy as np

P = 128
F, H, Dh, ED = 128, 8, 16, 16
FXE = 192          # bf16 elems per node block in a table row
ROWW = 2 * FXE     # pair row width (384 bf16 = 768B)
KCAP = 40          # max edge slot-cols per chunk
GMAX = 8           # slot-cols per gather instruction (1024 idxs)
EPS_IN, NEG, MNEG = 1e-5, 0.2, -1e30


def _cfg_full():
    return dict(N=50000, E=1600000, NC=8)


def _fold_weights(W, att_src, att_dst, W_e, att_edge):
    import ml_dtypes
    w_src = np.stack(
        [W[:, h * Dh:(h + 1) * Dh] @ att_src[h] for h in range(H)], axis=1)
    w_dst = np.stack(
        [W[:, h * Dh:(h + 1) * Dh] @ att_dst[h] for h in range(H)], axis=1)
    Wb = np.concatenate([W, w_src, w_dst], axis=1)  # [F, 144]
    v = np.stack(
        [W_e[:, h * Dh:(h + 1) * Dh] @ att_edge[h] for h in range(H)], axis=1)
    v8 = np.zeros((8 * ED, 8 * H), dtype=np.float32)
    for s in range(8):
        v8[s * ED:(s + 1) * ED, s * H:(s + 1) * H] = v
    return Wb.astype(ml_dtypes.bfloat16), v8.astype(ml_dtypes.bfloat16)


def _chunks_of(K):
    """Chunk list for a tile with K edge slots: [(j0, ne, has_self), ...].
    Chunk 0 (with the self col) is listed first; device processes it LAST."""
    ch = [(0, min(K, KCAP - 1), True)]
    j = KCAP - 1
    while j < K:
        ch.append((j, min(KCAP, K - j), False))
        j += KCAP
    return ch


def _pack16(flat):
    cols = len(flat) // 16
    out2 = np.zeros((P, max(cols, 1)), dtype=np.int16)
    if cols:
        out2[:] = np.tile(flat.reshape(-1, 16).T, (8, 1))
    return out2


def _preprocess(x, edge_index, edge_attr, cfg):
    import ml_dtypes
    N, E, NC = cfg["N"], cfg["E"], cfg["NC"]
    Np = N // NC
    n_tiles = math.ceil(Np / P)
    src = np.asarray(edge_index[0]).astype(np.int64)
    dst = np.asarray(edge_index[1]).astype(np.int64)
    ea = np.asarray(edge_attr, dtype=np.float32)
    x_np = np.asarray(x, dtype=np.float32)

    # ---- node -> (core, tile, partition): global-degree snake deal
    deg_g = np.bincount(dst, minlength=N)
    order = np.argsort(-deg_g, kind="stable")
    ranks = np.arange(N)
    blk, pos = ranks // NC, ranks % NC
    core_of_rank = np.where(blk % 2 == 0, pos, NC - 1 - pos)
    assign = np.empty(N, dtype=np.int64)
    assign[order] = core_of_rank
    local_rank = np.empty(N, dtype=np.int64)
    nodes_of_core = []
    for c in range(NC):
        nodes_c = order[core_of_rank == c]          # deg-desc order
        assert len(nodes_c) == Np
        local_rank[nodes_c] = np.arange(Np)
        nodes_of_core.append(nodes_c)

    # ---- per-core edge routing and per-tile max degree
    cores = []
    Kct = np.zeros((NC, n_tiles), dtype=np.int64)
    for c in range(NC):
        m = assign[dst] == c
        e_ids = np.nonzero(m)[0]
        dl = local_rank[dst[e_ids]]
        o = np.argsort(dl, kind="stable")
        e_ids, dl = e_ids[o], dl[o]
        deg = np.bincount(dl, minlength=Np)
        cum = np.zeros(Np + 1, dtype=np.int64)
        np.cumsum(deg, out=cum[1:])
        j_e = np.arange(len(dl)) - cum[dl]
        t_e, p_e = dl // P, dl % P
        np.maximum.at(Kct[c], t_e, j_e + 1)
        cores.append(dict(e_ids=e_ids, dl=dl, j=j_e, t=t_e, p=p_e, deg=deg))

    K_t = Kct.max(axis=0)

    # ---- shared chunk schedule + offsets (identical across cores)
    chunks = []          # (t, j0, ne, has_self, C, EQ)
    for t in range(n_tiles):
        for (j0, ne, hs) in _chunks_of(int(K_t[t])):
            C = ne + (1 if hs else 0)
            EQ = (ne + 7) // 8
            chunks.append((t, j0, ne, hs, C, EQ))
    n_chunks = len(chunks)
    idx_off = np.zeros(n_chunks + 1, dtype=np.int64)   # in idxs
    mask_off = np.zeros(n_chunks + 1, dtype=np.int64)  # in cols (per partition)
    ea_off = np.zeros(n_chunks + 1, dtype=np.int64)    # in cols
    for i, (t, j0, ne, hs, C, EQ) in enumerate(chunks):
        idx_off[i + 1] = idx_off[i] + ne * P
        mask_off[i + 1] = mask_off[i] + C * 16
        ea_off[i + 1] = ea_off[i] + EQ * P
    chunk_no_of = {}
    for i, (t, j0, ne, hs, C, EQ) in enumerate(chunks):
        chunk_no_of[(t, j0)] = i

    # chunk id lookup for an edge slot j: piecewise
    def _ci_arrays(j):
        in0 = j < (KCAP - 1)
        ci = np.where(in0, 0, 1 + (j - (KCAP - 1)) // KCAP)
        j0 = np.where(in0, 0, (KCAP - 1) + ((j - (KCAP - 1)) // KCAP) * KCAP)
        jj = j - j0
        cc = jj + np.where(in0, 1, 0)   # col within chunk (self col shifts)
        return ci, j0, jj, cc

    ea_bf = ea.astype(ml_dtypes.bfloat16)
    SIDX = int(idx_off[-1])
    SMASK = int(mask_off[-1])
    SEA = int(ea_off[-1])

    # tile-major base chunk numbers
    for c in range(NC):
        st = cores[c]
        t_e, p_e, j_e = st["t"], st["p"], st["j"]
        src_e = src[st["e_ids"]]
        ci, j0, jj, cc = _ci_arrays(j_e)
        # vectorized chunk_no: build lookup [n_tiles, max_ci]
        max_ci = 1 + max(0, (int(K_t.max()) - (KCAP - 1) + KCAP - 1) // KCAP)
        lut = np.full((n_tiles, max_ci + 1), -1, dtype=np.int64)
        for i, (t, jj0, ne, hs, C, EQ) in enumerate(chunks):
            cidx = 0 if hs else 1 + (jj0 - (KCAP - 1)) // KCAP
            lut[t, cidx] = i
        cno = lut[t_e, ci]
        assert (cno >= 0).all()

        idxA = np.zeros(SIDX, dtype=np.int16)
        idxA[idx_off[cno] + jj * P + p_e] = (src_e >> 1).astype(np.int16)
        maskA = np.full((P, SMASK), MNEG, dtype=ml_dtypes.bfloat16)
        colm = (mask_off[cno] + cc * 16 + (src_e & 1) * 8).astype(np.int64)
        maskA[p_e[:, None], colm[:, None] + np.arange(8)[None, :]] = 0.0
        # self cols: even half active
        for i, (t, jj0, ne, hs, C, EQ) in enumerate(chunks):
            if hs:
                maskA[:, int(mask_off[i]):int(mask_off[i]) + 8] = 0.0
        ea8 = np.zeros((8 * ED, SEA), dtype=ml_dtypes.bfloat16)
        q_e, s_e = jj // 8, jj % 8
        cole = (ea_off[cno] + q_e * P + p_e).astype(np.int64)
        rows = (s_e[:, None] * ED + np.arange(ED)[None, :]).astype(np.int64)
        ea8[rows, cole[:, None]] = ea_bf[st["e_ids"]]

        rdeg = np.ones((P, n_tiles), dtype=np.float32)
        deg = st["deg"]
        idxs = np.arange(Np)
        rdeg[idxs % P, idxs // P] = 1.0 / np.maximum(deg, 1.0)

        nodes_c = nodes_of_core[c]
        pad = n_tiles * P - Np
        xo = np.zeros((n_tiles * P, F), dtype=np.float32)
        xo[:Np] = x_np[nodes_c]
        xTo = np.ascontiguousarray(xo.T).astype(ml_dtypes.bfloat16)
        st["in"] = dict(idx=_pack16(idxA), mask=maskA, ea8=ea8, rdeg=rdeg,
                        xo=xo, xTo=xTo)

    # pair-interleaved xT for Phase A (shared by all cores); evens at
    # partitions 0..63, odds at 64..127 of each 128-node chunk, zero-padded
    n_chunksA = math.ceil(N / P)
    xpad = np.zeros((n_chunksA * P, F), dtype=np.float32)
    for i0 in range(0, N, P):
        nrow = min(P, N - i0)
        assert nrow % 2 == 0
        xpad[i0:i0 + nrow // 2] = x_np[i0:i0 + nrow:2]
        xpad[i0 + 64:i0 + 64 + nrow // 2] = x_np[i0 + 1:i0 + nrow:2]
    xT_pa = np.ascontiguousarray(xpad.T).astype(ml_dtypes.bfloat16)

    meta = dict(N=N, NC=NC, Np=Np, n_tiles=n_tiles, K_t=K_t, chunks=chunks,
                idx_off=idx_off, mask_off=mask_off, ea_off=ea_off,
                SIDX=SIDX, SMASK=SMASK, SEA=SEA)
    return cores, nodes_of_core, xT_pa, meta


# ---------------------------------------------------------------- device
def _build(meta, finalize=True):
    import concourse.bass as bass
    import concourse.bacc as bacc
    import concourse.tile as tile
    from concourse import mybir

    N, NC = meta["N"], meta["NC"]
    n_tiles = meta["n_tiles"]
    chunks = meta["chunks"]
    idx_off, mask_off, ea_off = meta["idx_off"], meta["mask_off"], meta["ea_off"]
    SIDX, SMASK, SEA = meta["SIDX"], meta["SMASK"], meta["SEA"]
    NPAIR = N // 2
    f32 = mybir.dt.float32
    bf16 = mybir.dt.bfloat16
    i16 = mybir.dt.int16
    AF = mybir.ActivationFunctionType
    OP = mybir.AluOpType
    KMAX = int(max(c[4] for c in chunks))   # max C

    n_chunksA = math.ceil(N / P)
    nc = bacc.Bacc("TRN2", target_bir_lowering=False, debug=False,
                   num_devices=NC)
    xT_d = nc.declare_dram_parameter("xT", [F, n_chunksA * P], bf16,
                                     isOutput=False)
    xTo_d = nc.declare_dram_parameter("xTo", [F, n_tiles * P], bf16,
                                      isOutput=False)
    xo_d = nc.declare_dram_parameter("xo", [n_tiles * P, F], f32,
                                     isOutput=False)
    Wb_d = nc.declare_dram_parameter("Wb", [F, 144], bf16, isOutput=False)
    v8_d = nc.declare_dram_parameter("v8", [8 * ED, 8 * H], bf16,
                                     isOutput=False)
    ix_d = nc.declare_dram_parameter("idx", [P, max(SIDX // 16, 1)], i16,
                                     isOutput=False)
    mk_d = nc.declare_dram_parameter("mask", [P, SMASK], bf16, isOutput=False)
    ea_d = nc.declare_dram_parameter("ea8", [8 * ED, SEA], bf16,
                                     isOutput=False)
    rdeg_d = nc.declare_dram_parameter("rdeg", [P, n_tiles], f32,
                                       isOutput=False)
    gam_d = nc.declare_dram_parameter("gamma", [F], f32, isOutput=False)
    bet_d = nc.declare_dram_parameter("beta", [F], f32, isOutput=False)
    out_d = nc.declare_dram_parameter("out", [n_tiles * P, F], f32,
                                      isOutput=True)

    with tile.TileContext(nc) as tc:
        with (
            tc.tile_pool(name="dram", bufs=1, space="DRAM") as dram,
            tc.tile_pool(name="consts", bufs=1) as consts,
            tc.tile_pool(name="keep", bufs=1) as keep,
        ):
            hx = dram.tile([NPAIR, ROWW], bf16)

            Wb_s = consts.tile([F, 144], bf16)
            nc.sync.dma_start(out=Wb_s[:], in_=Wb_d[:, :])
            v8_s = consts.tile([8 * ED, 8 * H], bf16)
            nc.sync.dma_start(out=v8_s[:], in_=v8_d[:, :])
            ones = consts.tile([P, 1], f32)
            nc.vector.memset(ones[:], 1.0)
            rdeg_s = consts.tile([P, n_tiles], f32)
            nc.sync.dma_start(out=rdeg_s[:], in_=rdeg_d[:, :])

            hx_own = keep.tile([P, n_tiles, 144], bf16)
            out_all = keep.tile([P, n_tiles, F], f32)
            acc = keep.tile([P, 2], f32)
            nc.vector.memset(acc[:], 0.0)

            # ---------------- Phase A: pair table hx = x @ Wb
            # 8 node-chunks share one staging tile -> 2 bulk DMAs per group
            with (
                tc.tile_pool(name="pha", bufs=6) as pha,
                tc.tile_pool(name="pha_ps", bufs=4, space="PSUM") as pha_ps,
            ):
                CB = 16
                n_full = N // P           # full 128-node chunks
                for i0 in range(0, n_chunksA, CB):
                    nb = min(CB, n_chunksA - i0)
                    bulk = (i0 + nb <= n_full)   # all chunks full-size
                    st8 = pha.tile([P, CB, FXE], bf16, name="st8", tag="st8")
                    nc.vector.memset(st8[:, :, 136:FXE], 0.0)
                    for j in range(0, nb, 8):
                        nx = min(8, nb - j)
                        xT_t = pha.tile([F, 8 * P], bf16, name="xT_t",
                                        tag="xT_t")
                        nc.scalar.dma_start(
                            out=xT_t[:, 0:nx * P],
                            in_=xT_d[:, (i0 + j) * P:(i0 + j + nx) * P])
                        for k in range(nx):
                            hp = pha_ps.tile([P, 144], f32, name="hp",
                                             tag="hp")
                            nc.tensor.matmul(out=hp[:],
                                             lhsT=xT_t[:, k * P:(k + 1) * P],
                                             rhs=Wb_s[:],
                                             start=True, stop=True)
                            nc.vector.tensor_copy(
                                out=st8[:, j + k, 0:136], in_=hp[:, 0:136])
                    prg = i0 * 64
                    if bulk:
                        nc.sync.dma_start(
                            out=hx[prg:prg + nb * 64, 0:FXE]
                                .rearrange("(c p) f -> p c f", p=64),
                            in_=st8[0:64, 0:nb, :])
                        nc.scalar.dma_start(
                            out=hx[prg:prg + nb * 64, FXE:ROWW]
                                .rearrange("(c p) f -> p c f", p=64),
                            in_=st8[64:128, 0:nb, :])
                    else:
                        for j in range(nb):
                            r0 = (i0 + j) * P
                            npair = min(P, N - r0) // 2
                            pr0 = r0 // 2
                            nc.sync.dma_start(
                                out=hx[pr0:pr0 + npair, 0:FXE],
                                in_=st8[0:npair, j, :])
                            nc.scalar.dma_start(
                                out=hx[pr0:pr0 + npair, FXE:ROWW],
                                in_=st8[64:64 + npair, j, :])
                # own nodes (tile order): h | asrc | adst, fp32
                for t in range(n_tiles):
                    xTo_t = pha.tile([F, P], bf16, name="xTo_t", tag="xT_t")
                    nc.sync.dma_start(out=xTo_t[:],
                                      in_=xTo_d[:, t * P:(t + 1) * P])
                    hp = pha_ps.tile([P, 144], f32, name="hp2", tag="hp")
                    nc.tensor.matmul(out=hp[:], lhsT=xTo_t[:], rhs=Wb_s[:],
                                     start=True, stop=True)
                    nc.vector.tensor_copy(out=hx_own[:, t, :], in_=hp[:])

            # ---------------- Phase B: per-tile attention + aggregation
            with (
                tc.tile_pool(name="phb", bufs=3) as phb,
                tc.tile_pool(name="acc_p", bufs=2) as accp,
                tc.tile_pool(name="phb_ps", bufs=4, space="PSUM") as phb_ps,
                tc.tile_pool(name="st_ps", bufs=2, space="PSUM") as st_ps,
            ):
                for t in range(n_tiles):
                    tile_chunks = [ch for ch in chunks if ch[0] == t]
                    # process non-self chunks first, self chunk last
                    tile_chunks = ([c for c in tile_chunks if not c[3]]
                                   + [c for c in tile_chunks if c[3]])
                    msg_acc = accp.tile([P, F], f32, name="msg_acc",
                                        tag="msg_acc")
                    den_acc = accp.tile([P, H], f32, name="den_acc",
                                        tag="den_acc")
                    aeL_acc = accp.tile([P, H], f32, name="aeL_acc",
                                        tag="aeL_acc")
                    nc.vector.memset(msg_acc[:], 0.0)
                    nc.vector.memset(den_acc[:], 0.0)
                    nc.vector.memset(aeL_acc[:], 0.0)
                    for (tt, j0, ne, hs, C, EQ) in tile_chunks:
                        cno = None
                        for i, ch in enumerate(chunks):
                            if ch[0] == t and ch[1] == j0:
                                cno = i
                                break
                        e0 = 1 if hs else 0
                        g = phb.tile([P, KMAX, ROWW], bf16, name="g", tag="g")
                        mk = phb.tile([P, KMAX, 16], bf16, name="mk", tag="mk")
                        nc.scalar.dma_start(
                            out=mk[:, 0:C, :].rearrange("p c h -> p (c h)"),
                            in_=mk_d[:, int(mask_off[cno]):int(mask_off[cno + 1])])
                        if ne:
                            ixt = phb.tile([P, KCAP * 8], i16, name="ixt",
                                           tag="ixt")
                            o0 = int(idx_off[cno]) // 16
                            nc.scalar.dma_start(out=ixt[:, 0:ne * 8],
                                                in_=ix_d[:, o0:o0 + ne * 8])
                            ea8 = phb.tile([8 * ED, (KCAP // 8) * P], bf16,
                                           name="ea8", tag="ea8")
                            nc.sync.dma_start(
                                out=ea8[:, 0:EQ * P],
                                in_=ea_d[:, int(ea_off[cno]):int(ea_off[cno + 1])])
                        if hs:
                            nc.vector.tensor_copy(out=g[:, 0, 0:136],
                                                  in_=hx_own[:, t, 0:136])
                            nc.vector.tensor_copy(out=g[:, 0, FXE:FXE + 136],
                                                  in_=hx_own[:, t, 0:136])
                        for g0 in range(0, ne, GMAX):
                            kk = min(GMAX, ne - g0)
                            nc.gpsimd.dma_gather(
                                out_ap=g[:, e0 + g0:e0 + g0 + kk, :],
                                in_ap=hx[:, :],
                                idxs_ap=ixt[:, g0 * 8:(g0 + kk) * 8],
                                num_idxs=kk * P,
                                num_idxs_reg=kk * P,
                                elem_size=ROWW,
                            )
                        # a_edge
                        ae = phb.tile([P, KMAX, H], f32, name="ae", tag="ae")
                        for q in range(EQ):
                            aep = phb_ps.tile([P, 8 * H], f32, name="aep",
                                              tag="aep")
                            nc.tensor.matmul(
                                out=aep[:],
                                lhsT=ea8[:, q * P:(q + 1) * P],
                                rhs=v8_s[:], start=True, stop=True)
                            nq = min(8, ne - q * 8)
                            nc.vector.tensor_copy(
                                out=ae[:, e0 + q * 8:e0 + q * 8 + nq, :]
                                    .rearrange("p c h -> p (c h)"),
                                in_=aep[:, 0:nq * H])
                        if ne:
                            aeL = phb.tile([P, H], f32, name="aeL", tag="aeL")
                            nc.vector.tensor_reduce(
                                out=aeL[:],
                                in_=ae[:, e0:e0 + ne, :].transpose([0, 2, 1]),
                                axis=mybir.AxisListType.X, op=OP.add)
                            nc.vector.tensor_add(aeL_acc[:], aeL_acc[:],
                                                 aeL[:])
                        if hs:
                            nc.vector.tensor_scalar_mul(
                                ae[:, 0, :], aeL_acc[:], rdeg_s[:, t:t + 1])
                        # logits [P, C, 16] fp32
                        al = phb.tile([P, KMAX, 16], f32, name="al", tag="al")
                        adst_b = hx_own[:, t, 136:144].unsqueeze(1) \
                            .broadcast_to((P, C, H))
                        nc.vector.tensor_tensor(
                            out=al[:, 0:C, 0:8], in0=ae[:, 0:C, :],
                            in1=adst_b, op=OP.add)
                        nc.vector.tensor_tensor(
                            out=al[:, 0:C, 8:16], in0=ae[:, 0:C, :],
                            in1=adst_b, op=OP.add)
                        nc.vector.tensor_tensor(
                            out=al[:, 0:C, 0:8], in0=al[:, 0:C, 0:8],
                            in1=g[:, 0:C, 128:136], op=OP.add)
                        nc.vector.tensor_tensor(
                            out=al[:, 0:C, 8:16], in0=al[:, 0:C, 8:16],
                            in1=g[:, 0:C, FXE + 128:FXE + 136], op=OP.add)
                        nc.vector.tensor_tensor(
                            out=al[:, 0:C, :], in0=al[:, 0:C, :],
                            in1=mk[:, 0:C, :], op=OP.add)
                        nc.vector.scalar_tensor_tensor(
                            out=al[:, 0:C, :], in0=al[:, 0:C, :], scalar=NEG,
                            in1=al[:, 0:C, :], op0=OP.mult, op1=OP.max)
                        nc.vector.tensor_scalar_max(al[:, 0:C, :],
                                                    al[:, 0:C, :], -88.0)
                        nc.scalar.activation(out=g[:, 0:C, 136:152],
                                             in_=al[:, 0:C, :], func=AF.Exp)
                        # fold alpha into h (both halves)
                        nc.vector.tensor_tensor(
                            out=g[:, 0:C, 0:128].rearrange(
                                "p c (h d) -> p c h d", h=H),
                            in0=g[:, 0:C, 0:128].rearrange(
                                "p c (h d) -> p c h d", h=H),
                            in1=g[:, 0:C, 136:144].unsqueeze(3)
                                .broadcast_to((P, C, H, Dh)),
                            op=OP.mult)
                        nc.vector.tensor_tensor(
                            out=g[:, 0:C, FXE:FXE + 128].rearrange(
                                "p c (h d) -> p c h d", h=H),
                            in0=g[:, 0:C, FXE:FXE + 128].rearrange(
                                "p c (h d) -> p c h d", h=H),
                            in1=g[:, 0:C, 144:152].unsqueeze(3)
                                .broadcast_to((P, C, H, Dh)),
                            op=OP.mult)
                        # halving trees: even block (+asrc junk+alpha/den),
                        # odd block
                        c = C
                        while c > 1:
                            hh = c // 2
                            nc.vector.tensor_tensor(
                                out=g[:, 0:hh, 0:152], in0=g[:, 0:hh, 0:152],
                                in1=g[:, c - hh:c, 0:152], op=OP.add)
                            nc.vector.tensor_tensor(
                                out=g[:, 0:hh, FXE:FXE + 128],
                                in0=g[:, 0:hh, FXE:FXE + 128],
                                in1=g[:, c - hh:c, FXE:FXE + 128], op=OP.add)
                            c -= hh
                        nc.vector.tensor_add(msg_acc[:], msg_acc[:],
                                             g[:, 0, 0:128])
                        nc.vector.tensor_add(msg_acc[:], msg_acc[:],
                                             g[:, 0, FXE:FXE + 128])
                        nc.vector.tensor_add(den_acc[:], den_acc[:],
                                             g[:, 0, 136:144])
                        nc.vector.tensor_add(den_acc[:], den_acc[:],
                                             g[:, 0, 144:152])
                    # normalize + stats
                    rec = accp.tile([P, H], f32, name="rec", tag="rec")
                    nc.vector.tensor_scalar_add(rec[:], den_acc[:], 1e-16)
                    nc.vector.reciprocal(rec[:], rec[:])
                    op_t = out_all[:, t, :]
                    nc.vector.tensor_tensor(
                        out=op_t.rearrange("p (h d) -> p h d", h=H),
                        in0=msg_acc.rearrange("p (h d) -> p h d", h=H),
                        in1=rec.unsqueeze(2).broadcast_to((P, H, Dh)),
                        op=OP.mult)
                    sq = accp.tile([P, F], f32, name="sq", tag="sq")
                    nc.vector.tensor_mul(sq[:], op_t, op_t)
                    stp = st_ps.tile([P, 2], f32, name="stp", tag="stp")
                    nc.tensor.matmul(out=stp[:, 0:1], lhsT=op_t, rhs=ones[:],
                                     start=True, stop=True)
                    nc.tensor.matmul(out=stp[:, 1:2], lhsT=sq[:], rhs=ones[:],
                                     start=True, stop=True)
                    nc.vector.tensor_add(acc[:], acc[:], stp[:])

            # ---------------- Phase C: stats allreduce + normalize + ELU
            st_in = dram.tile([P, 2], f32)
            st_out = dram.tile([P, 2], f32, addr_space="Shared")
            nc.sync.dma_start(out=st_in[:], in_=acc[:])
            nc.gpsimd.collective_compute(
                "AllReduce", mybir.AluOpType.add,
                replica_groups=[list(range(NC))],
                ins=[st_in[:].opt()], outs=[st_out[:].opt()])
            sg = keep.tile([P, 2], f32)
            nc.sync.dma_start(out=sg[:], in_=st_out[:])
            mean = keep.tile([P, 1], f32)
            nc.vector.tensor_scalar_mul(mean[:], sg[:, 0:1], 1.0 / N)
            ex2 = keep.tile([P, 1], f32)
            nc.vector.tensor_scalar_mul(ex2[:], sg[:, 1:2], 1.0 / N)
            var = keep.tile([P, 1], f32)
            nc.vector.tensor_mul(var[:], mean[:], mean[:])
            nc.vector.tensor_sub(var[:], ex2[:], var[:])
            rstd = keep.tile([P, 1], f32)
            eps_t = keep.tile([P, 1], f32)
            nc.vector.memset(eps_t[:], EPS_IN)
            nc.scalar.activation(out=rstd[:], in_=var[:], func=AF.Sqrt,
                                 bias=eps_t[:])
            nc.vector.reciprocal(rstd[:], rstd[:])
            gam_s = keep.tile([P, 1], f32)
            nc.sync.dma_start(out=gam_s[:], in_=gam_d[:, None])
            bet_s = keep.tile([P, 1], f32)
            nc.sync.dma_start(out=bet_s[:], in_=bet_d[:, None])
            scl = keep.tile([P, 1], f32)
            nc.vector.tensor_mul(scl[:], rstd[:], gam_s[:])
            bia = keep.tile([P, 1], f32)
            nc.vector.tensor_mul(bia[:], mean[:], scl[:])
            nc.vector.tensor_sub(bia[:], bet_s[:], bia[:])
            sb_dram = dram.tile([2, P], f32)
            nc.sync.dma_start(out=sb_dram[0, :], in_=scl[:, 0])
            nc.sync.dma_start(out=sb_dram[1, :], in_=bia[:, 0])
            sclB = keep.tile([P, F], f32)
            nc.sync.dma_start(out=sclB[:],
                              in_=sb_dram[0:1, :].broadcast_to((P, P)))
            biaB = keep.tile([P, F], f32)
            nc.sync.dma_start(out=biaB[:],
                              in_=sb_dram[1:2, :].broadcast_to((P, P)))

            with tc.tile_pool(name="phc", bufs=2) as phc:
                TB = 25
                for t0 in range(0, n_tiles, TB):
                    nt = min(TB, n_tiles - t0)
                    xo_t = phc.tile([P, TB, F], f32, name="xo_t")
                    nc.sync.dma_start(
                        out=xo_t[:, 0:nt, :],
                        in_=xo_d[t0 * P:(t0 + nt) * P, :]
                            .rearrange("(c p) f -> p c f", p=P))
                    z = phc.tile([P, TB, F], f32, name="z")
                    nc.vector.tensor_tensor(
                        out=z[:, 0:nt, :], in0=out_all[:, t0:t0 + nt, :],
                        in1=sclB.unsqueeze(1).broadcast_to((P, nt, F)),
                        op=OP.mult)
                    nc.vector.tensor_tensor(
                        out=z[:, 0:nt, :], in0=z[:, 0:nt, :],
                        in1=biaB.unsqueeze(1).broadcast_to((P, nt, F)),
                        op=OP.add)
                    nc.vector.tensor_add(z[:, 0:nt, :], z[:, 0:nt, :],
                                         xo_t[:, 0:nt, :])
                    zf = z[:, 0:nt, :].rearrange("p c f -> p (c f)")
                    zm = phc.tile([P, TB, F], f32, name="zm")
                    zmf = zm[:, 0:nt, :].rearrange("p c f -> p (c f)")
                    nc.vector.tensor_scalar_min(zmf, zf, 0.0)
                    nc.scalar.activation(out=zmf, in_=zmf, func=AF.Exp)
                    nc.vector.tensor_scalar_max(zf, zf, 0.0)
                    nc.vector.tensor_add(zf, zf, zmf)
                    nc.vector.tensor_scalar_add(zf, zf, -1.0)
                    nc.scalar.dma_start(
                        out=out_d[t0 * P:(t0 + nt) * P, :]
                            .rearrange("(c p) f -> p c f", p=P),
                        in_=z[:, 0:nt, :])
    if finalize:
        nc.finalize()
    return nc


# ---------------------------------------------------------------- driver
def _run_gat(x, edge_index, edge_attr, W, att_src, att_dst, W_e, att_edge,
             gamma, beta, cfg, trace=False, return_results=False, sim=False):
    N, NC = cfg["N"], cfg["NC"]
    Np = N // NC
    Wb, v8 = _fold_weights(
        np.asarray(W, np.float32), np.asarray(att_src, np.float32),
        np.asarray(att_dst, np.float32), np.asarray(W_e, np.float32),
        np.asarray(att_edge, np.float32))
    cores, nodes_of_core, xT_pa, meta = _preprocess(x, edge_index, edge_attr,
                                                    cfg)
    nc = _build(meta)

    gam = np.asarray(gamma, np.float32)
    bet = np.asarray(beta, np.float32)
    n_tiles = meta["n_tiles"]
    in_maps = []
    for c in range(NC):
        sti = cores[c]["in"]
        in_maps.append(dict(
            xT=xT_pa, xTo=sti["xTo"], xo=sti["xo"], Wb=Wb, v8=v8,
            idx=sti["idx"], mask=sti["mask"], ea8=sti["ea8"],
            rdeg=sti["rdeg"], gamma=gam, beta=bet))
    if sim:
        from concourse.bass_interp import MultiCoreSim
        ms = MultiCoreSim(nc, num_cores=NC)
        for c, cs in ms.cores.items():
            for k, v in in_maps[c].items():
                cs.tensor(k)[:] = v
        ms.simulate()
        results = [{"out": np.asarray(ms.cores[c].tensor("out"))}
                   for c in range(NC)]
        res = None
    else:
        from concourse.bass_utils import run_bass_kernel_spmd
        res = run_bass_kernel_spmd(nc, in_maps, core_ids=list(range(NC)),
                                   trace=trace)
        results = res.results
    out = np.empty((N, F), dtype=np.float32)
    for c in range(NC):
        oc = results[c]["out"]
        out[nodes_of_core[c]] = oc[:Np]
    if return_results:
        return out, res
    return out


def kernel(x, edge_index, edge_attr, W, att_src, att_dst, W_e, att_edge,
           gamma, beta):
    return _run_gat(x, edge_index, edge_attr, W, att_src, att_dst, W_e,
                    att_edge, gamma, beta, _cfg_full())
